# revision 18
# baseline (speedup 1.0000x reference)
"""Multi-head attention (B=4, S=2048, H=8, Dh=64, Dm=512) on 8 TRN2 NeuronCores.

Sharding: batch*head parallel. Core c owns batch b = c//2 and head group
g = c%2 (4 heads each). Each core computes QKV projection for its head
group, transposed-scores flash-style attention (no max subtraction --
scores ~ N(0,1) after 1/sqrt(Dh) scaling, exp is safe in fp32/bf16), and
its partial output projection against its 256 rows of Wo. The host sums
the two partial projections per batch.

Device-side layout notes:
  - X^T (bf16) is prepared on host so every matmul contracts over the
    partition dim directly.
  - Scores are computed transposed (S^T[j,i] = K Q^T) so the attention*V
    matmul needs no transposition; the two heads of a 128-row Q^T/K^T
    chunk are packed into the PE array as two K=64 row-tiles (tile_position
    (0,0)/(64,0)) running concurrently.
  - Row sums of exp(scores) come for free from a ones-column appended to V
    (M=65 stationary); normalization uses a K=1 broadcast matmul + DVE
    reciprocal/multiply.
"""

import os
import sys

for _p in ("/opt/trn_rl_repo",):
    if os.path.isdir(_p) and _p not in sys.path:
        sys.path.append(_p)

import ml_dtypes
import numpy as np

import concourse.bass as bass
import concourse.tile as tile
from concourse import bacc, mybir
from concourse.bass_utils import run_bass_kernel_spmd

BF16 = mybir.dt.bfloat16
F32 = mybir.dt.float32

B, S, DM = 4, 2048, 512
H, DH = 8, 64
HPC = 4  # heads per core
DQ = HPC * DH  # 256: per-core slice of the inner dim
N_CORES = 8
SCALE = DH**-0.5

AF = mybir.ActivationFunctionType

# exported for test harnesses
LAST_EXEC_TIME_NS = None
LAST_RESULT = None

_CACHED_NC = None


def _kernel_body(tc, xT_d, wq_d, wk_d, wv_d, wo_d, out_d):
    from contextlib import ExitStack

    nc = tc.nc
    with ExitStack() as ctx:
        consts = ctx.enter_context(tc.tile_pool(name="consts", bufs=1))
        ptp = ctx.enter_context(tc.tile_pool(name="pt", bufs=4))
        normp = ctx.enter_context(tc.tile_pool(name="norm", bufs=2))
        foutp = ctx.enter_context(tc.tile_pool(name="fout", bufs=3))
        # PSUM: "s" 2x[128,1024]=4 banks, "o" 2x[65,512]=2, "b" 1, "x" 1 -> 8
        ps_s = ctx.enter_context(tc.tile_pool(name="ps_s", bufs=2, space="PSUM"))
        ps_o = ctx.enter_context(tc.tile_pool(name="ps_o", bufs=3, space="PSUM"))
        ps_x = ctx.enter_context(tc.tile_pool(name="ps_x", bufs=1, space="PSUM"))
        drp = ctx.enter_context(tc.tile_pool(name="dram", bufs=2, space="DRAM"))

        sb_xT = consts.tile([128, 4, S], BF16)  # X^T: k-chunk c -> [:, c, :]
        sb_wq = consts.tile([128, 4, DQ], BF16)
        sb_wk = consts.tile([128, 4, DQ], BF16)
        sb_wv = consts.tile([128, 4, DQ], BF16)
        sb_wo = consts.tile([128, 2, DM], BF16)  # d'-chunk p -> [:, p, :]
        sb_qT = consts.tile([128, 2, S], BF16)  # dq-chunk (head pair) p
        sb_kT = consts.tile([128, 2, S], BF16)
        sb_v = consts.tile([128, 16, HPC, 66], BF16)  # V_aug; col 64 = ones
        sb_oT = consts.tile([128, 2, S], BF16)  # normalized O^T
        sb_warm = consts.tile([128, 512], BF16)  # PE warmup fodder

        nc.vector.memset(sb_v[:, :, :, 64:66], 1.0)
        nc.vector.memset(sb_warm[:], 1.0)
        for w_d, w_sb in ((wq_d, sb_wq), (wk_d, sb_wk), (wv_d, sb_wv)):
            nc.sync.dma_start(w_sb[:], w_d.rearrange("(c p) d -> p c d", p=128))
        nc.sync.dma_start(sb_wo[:], wo_d.rearrange("(c p) d -> p c d", p=128))
        xT_r = xT_d.rearrange("(c p) s -> c p s", p=128)
        for kc in range(4):
            nc.sync.dma_start(sb_xT[:, kc, :], xT_r[kc])

        # Warm the PE (HAM un-throttle needs ~3.4us of sustained matmul) and
        # preload the exp table while the xT DMA is in flight. Enough dummy
        # matmuls to keep PE busy until the DMA lands (else the MID window
        # re-throttles it right before the real work starts).
        pw = ps_x.tile([128, 512], F32, tag="x")
        for r in range(24):
            nc.tensor.matmul(
                pw[:], lhsT=sb_warm[:, 0:128], rhs=sb_warm[:], start=True, stop=True
            )
        warm_act = normp.tile([1, 4], F32, tag="wact")
        nc.scalar.activation(warm_act[:], pw[0:1, 0:4], AF.Exp, scale=-1.0)

        def emit_qk_chunk(w_sb, dst_sb, p, c):
            """One [128,512] chunk of Q^T or K^T for head-pair p."""
            isl = slice(c * 512, (c + 1) * 512)
            pq = ps_s.tile([128, 512], F32, tag="s", name="pqk")
            for kc in range(4):
                nc.tensor.matmul(
                    pq[:],
                    lhsT=w_sb[:, kc, p * 128 : (p + 1) * 128],
                    rhs=sb_xT[:, kc, isl],
                    start=(kc == 0),
                    stop=(kc == 3),
                )
            nc.vector.tensor_copy(dst_sb[:, p, isl], pq[:])

        def emit_qk_chunk_mm(w_sb, p, c, kc, pq):
            nc.tensor.matmul(
                pq[:],
                lhsT=w_sb[:, kc, p * 128 : (p + 1) * 128],
                rhs=sb_xT[:, kc, c * 512 : (c + 1) * 512],
                start=(kc == 0),
                stop=(kc == 3),
            )

        def emit_v_chunk(sc):
            """V natural [s,dv] for s-chunk sc (all 4 heads)."""
            pv = ps_x.tile([128, DQ], F32, tag="x", name="pv")
            for kc in range(4):
                nc.tensor.matmul(
                    pv[:],
                    lhsT=sb_xT[:, kc, sc * 128 : (sc + 1) * 128],
                    rhs=sb_wv[:, kc, :],
                    start=(kc == 0),
                    stop=(kc == 3),
                )
            nc.vector.tensor_copy(
                sb_v[:, sc, :, 0:64], pv.rearrange("p (h d) -> p h d", h=HPC)
            )

        def emit_proj_chunk(c2, tag="x"):
            pf = ps_x.tile([128, 512], F32, tag=tag, name="pf") if tag == "x" else (
                ps_o.tile([128, 512], F32, tag=tag, name="pf2")
            )
            for p in range(2):
                nc.tensor.matmul(
                    pf[:],
                    lhsT=sb_oT[:, p, c2 * 128 : (c2 + 1) * 128],
                    rhs=sb_wo[:, p, :],
                    start=(p == 0),
                    stop=(p == 1),
                )
            fo = foutp.tile([128, 512], F32, tag="fo")
            nc.vector.tensor_copy(fo[:], pf[:])
            nc.sync.dma_start(out_d[c2 * 128 : (c2 + 1) * 128, :], fo[:])

        # ---- lead: Q^T/K^T for pair 0, first two V chunks ----
        for c in range(4):
            emit_qk_chunk(sb_wq, sb_qT, 0, c)
        for c in range(4):
            emit_qk_chunk(sb_wk, sb_kT, 0, c)
        emit_v_chunk(0)
        emit_v_chunk(1)

        # deferred work interleaved into attention blocks, one MM per j-iter
        pending_qk = []  # (w_sb, dst_sb, p, c) flattened to per-MM granularity
        for c in range(4):
            pending_qk.append((sb_wk, sb_kT, 1, c))
        for c in range(4):
            pending_qk.append((sb_wq, sb_qT, 1, c))
        qk_state = {"chunk": None, "tile": None, "kc": 0}

        def step_pending_qk():
            stt = qk_state
            if stt["chunk"] is None:
                if not pending_qk:
                    return
                stt["chunk"] = pending_qk.pop(0)
                stt["tile"] = ps_x.tile([128, 512], F32, tag="x", name="pqk1")
                stt["kc"] = 0
            w_sb, dst_sb, p, c = stt["chunk"]
            emit_qk_chunk_mm(w_sb, p, c, stt["kc"], stt["tile"])
            stt["kc"] += 1
            if stt["kc"] == 4:
                nc.vector.tensor_copy(
                    dst_sb[:, p, c * 512 : (c + 1) * 512], stt["tile"][:]
                )
                stt["chunk"] = None

        # ---- attention: pair 0 then pair 1 ----
        # Normalization of block k is emitted lazily, interleaved into the
        # first iterations of block k+1, so the in-order PE stream never
        # stalls long enough for HAM to re-throttle the clock.
        def emit_normalize(p, ic, po):
            """Normalize block (p, ic): 1/rowsum via DVE reciprocal + DMA
            stride-0 broadcast + DVE multiply. No PE instructions at all, so
            the in-order PE stream flows straight into the next block."""
            isl = slice(ic * 512, (ic + 1) * 512)
            for hi in range(2):
                # DVE lanes are hardwired to partitions (and the custom-DVE
                # reciprocal requires base-partition 0), so: copy the row-64
                # sums at base 64, bounce through DRAM, stride-0-broadcast to
                # base 0, then reciprocal there.
                sums = normp.tile([65, 512], F32, tag="sums", name=f"sums{hi}")
                nc.vector.tensor_copy(sums[64:65, :], po[hi][64:65, :])
                rd = drp.tile([1, 512], F32, name=f"rd{hi}")
                nc.sync.dma_start(rd[:], sums[64:65, :])
                brd = normp.tile([64, 512], F32, tag="brd", name=f"brd{hi}")
                nc.sync.dma_start(brd[:], rd[0:1, :].to_broadcast([64, 512]))
                rec = normp.tile([64, 512], F32, tag="rec", name=f"rec{hi}")
                nc.vector.reciprocal_approx_fast(rec[:], brd[:])
                if hi == 0:
                    nc.vector.tensor_mul(
                        sb_oT[0:64, p, isl], po[0][0:64, :], rec[:]
                    )
                else:
                    tmpb = normp.tile([64, 512], BF16, tag="tmpb")
                    nc.vector.tensor_mul(tmpb[:], po[1][0:64, :], rec[:])
                    nc.sync.dma_start(sb_oT[64:128, p, isl], tmpb[:])

        # per-MM-granularity deferred projection chunks (run during p1 blocks)
        pending_proj = []
        proj_state = {"c2": None, "tile": None, "p": 0}

        def step_pending_proj():
            stt = proj_state
            if stt["c2"] is None:
                if not pending_proj:
                    return
                stt["c2"] = pending_proj.pop(0)
                stt["tile"] = ps_x.tile([128, 512], F32, tag="x", name="pf")
                stt["p"] = 0
            c2, p = stt["c2"], stt["p"]
            nc.tensor.matmul(
                stt["tile"][:],
                lhsT=sb_oT[:, p, c2 * 128 : (c2 + 1) * 128],
                rhs=sb_wo[:, p, :],
                start=(p == 0),
                stop=(p == 1),
            )
            stt["p"] += 1
            if stt["p"] == 2:
                fo = foutp.tile([128, 512], F32, tag="fo")
                nc.vector.tensor_copy(fo[:], stt["tile"][:])
                nc.sync.dma_start(out_d[c2 * 128 : (c2 + 1) * 128, :], fo[:])
                stt["c2"] = None

        for p in range(2):
            for ic in range(4):
                isl = slice(ic * 512, (ic + 1) * 512)
                po = [
                    ps_o.tile([65, 512], F32, tag="o", name=f"po{hi}")
                    for hi in range(2)
                ]
                for j in range(16):
                    jsl = slice(j * 128, (j + 1) * 128)
                    # extras: one deferred matmul per iteration keeps PE fed
                    # without outrunning ACT
                    if p == 0 and ic == 0:
                        if j < 14:
                            emit_v_chunk(j + 2)
                    elif p == 0:
                        step_pending_qk()
                    else:
                        step_pending_proj()
                    st = ps_s.tile([128, 1024], F32, tag="s")
                    # two K=64 row-tiles run concurrently in the PE array
                    nc.tensor.matmul(
                        st[:, 0:512],
                        lhsT=sb_kT[0:64, p, jsl],
                        rhs=sb_qT[0:64, p, isl],
                        start=True,
                        stop=True,
                    )
                    nc.tensor.matmul(
                        st[:, 512:1024],
                        lhsT=sb_kT[64:128, p, jsl],
                        rhs=sb_qT[64:128, p, isl],
                        start=True,
                        stop=True,
                    )
                    pt = ptp.tile([128, 1024], BF16, tag="pt")
                    nc.scalar.activation(pt[:], st[:], AF.Exp, scale=SCALE)
                    for hi in range(2):
                        nc.tensor.matmul(
                            po[hi][:],
                            lhsT=sb_v[:, j, 2 * p + hi, 0:65],
                            rhs=pt[:, hi * 512 : (hi + 1) * 512],
                            start=(j == 0),
                            stop=(j == 15),
                            skip_group_check=True,
                        )
                emit_normalize(p, ic, po)
                if p == 1 and ic > 0:
                    pending_proj.extend(range(4 * (ic - 1), 4 * ic))

        # ---- tail: remaining projection chunks ----
        while pending_proj or proj_state["c2"] is not None:
            step_pending_proj()
        for c2 in range(12, 16):
            emit_proj_chunk(c2, tag="o" if c2 % 2 else "x")


def _build():
    nc = bacc.Bacc("TRN2", target_bir_lowering=False, debug=False, num_devices=N_CORES)
    xT = nc.dram_tensor("xT", [DM, S], BF16, kind="ExternalInput")
    wq = nc.dram_tensor("wq", [DM, DQ], BF16, kind="ExternalInput")
    wk = nc.dram_tensor("wk", [DM, DQ], BF16, kind="ExternalInput")
    wv = nc.dram_tensor("wv", [DM, DQ], BF16, kind="ExternalInput")
    wo = nc.dram_tensor("wo", [DQ, DM], BF16, kind="ExternalInput")
    out = nc.dram_tensor("out", [S, DM], F32, kind="ExternalOutput")
    with tile.TileContext(nc) as tc:
        _kernel_body(tc, xT.ap(), wq.ap(), wk.ap(), wv.ap(), wo.ap(), out.ap())
    nc.compile()
    return nc


def get_nc():
    global _CACHED_NC
    if _CACHED_NC is None:
        _CACHED_NC = _build()
    return _CACHED_NC


def _in_maps(hidden_states, Wq, Wk, Wv, Wo):
    bf = ml_dtypes.bfloat16
    maps = []
    for c in range(N_CORES):
        b, g = c // 2, c % 2
        cols = slice(g * DQ, (g + 1) * DQ)
        maps.append(
            {
                "xT": np.ascontiguousarray(hidden_states[b].T).astype(bf),
                "wq": np.ascontiguousarray(Wq[:, cols]).astype(bf),
                "wk": np.ascontiguousarray(Wk[:, cols]).astype(bf),
                "wv": np.ascontiguousarray(Wv[:, cols]).astype(bf),
                "wo": np.ascontiguousarray(Wo[cols, :]).astype(bf),
            }
        )
    return maps


def _ensure_profile_support():
    """Best-effort: register the axon NTFF profiling hook + defang the
    bucket upload (zero-egress container). Without this, trace=True dies
    on a missing ``antenv.axon_hooks`` module in this image."""
    import types

    try:
        import antenv

        if "antenv.axon_hooks" not in sys.modules:
            mod = types.ModuleType("antenv.axon_hooks")
            _h = {"hook": None}
            mod.set_axon_ntff_profile_hook = lambda h: _h.__setitem__("hook", h)
            mod.get_axon_ntff_profile_hook = lambda: _h["hook"]
            sys.modules["antenv.axon_hooks"] = mod
            antenv.axon_hooks = mod
        import antenv.axon_hooks as ah

        if ah.get_axon_ntff_profile_hook() is None:
            if "/root/.axon_site" not in sys.path:
                sys.path.append("/root/.axon_site")
            from trn_agent_boot.trn_boot import _ntff_profile_via_ctypes

            hook = _ntff_profile_via_ctypes("/opt/axon/libaxon_pjrt.so")
            if hook is not None:
                ah.set_axon_ntff_profile_hook(hook)
    except Exception:
        pass
    try:
        import concourse.bass_utils as bu

        bu.upload_artifacts = lambda tmpdir: tmpdir
    except Exception:
        pass


def kernel(hidden_states, Wq, Wk, Wv, Wo):
    global LAST_EXEC_TIME_NS, LAST_RESULT
    hidden_states = np.asarray(hidden_states, dtype=np.float32)
    Wq, Wk, Wv, Wo = (np.asarray(w, dtype=np.float32) for w in (Wq, Wk, Wv, Wo))

    trace = bool(os.environ.get("BASS_TRACE"))
    if trace:
        _ensure_profile_support()
    nc = get_nc()
    maps = _in_maps(hidden_states, Wq, Wk, Wv, Wo)
    res = run_bass_kernel_spmd(
        nc,
        maps,
        core_ids=list(range(N_CORES)),
        trace=trace,
        tmpdir=os.environ.get("BASS_TRACE_DIR") or None,
    )
    LAST_RESULT = res
    LAST_EXEC_TIME_NS = res.exec_time_ns

    out = np.empty((B, S, DM), dtype=np.float32)
    for b in range(B):
        out[b] = res.results[2 * b]["out"] + res.results[2 * b + 1]["out"]
    return out


if __name__ == "__main__":
    rng = np.random.default_rng(0)
    hs = rng.standard_normal((B, S, DM), dtype=np.float32)
    ws = [
        (rng.standard_normal((DM, DM), dtype=np.float32) / np.sqrt(DM))
        for _ in range(4)
    ]
    o = kernel(hs, *ws)
    print("out", o.shape, o.dtype, float(np.abs(o).mean()))
    print("exec_time_ns", LAST_EXEC_TIME_NS)


# revision 19
# speedup vs baseline: 1.1018x; 1.1018x over previous
"""Multi-head attention (B=4, S=2048, H=8, Dh=64, Dm=512) on 8 TRN2 NeuronCores.

Sharding: batch*head parallel. Core c owns batch b = c//2 and head group
g = c%2 (4 heads each). Each core computes QKV projection for its head
group, transposed-scores flash-style attention (no max subtraction --
scores ~ N(0,1) after 1/sqrt(Dh) scaling, exp is safe in fp32/bf16), and
its partial output projection against its 256 rows of Wo. The host sums
the two partial projections per batch.

Device-side layout notes:
  - X^T (bf16) is prepared on host so every matmul contracts over the
    partition dim directly.
  - Scores are computed transposed (S^T[j,i] = K Q^T) so the attention*V
    matmul needs no transposition; the two heads of a 128-row Q^T/K^T
    chunk are packed into the PE array as two K=64 row-tiles (tile_position
    (0,0)/(64,0)) running concurrently.
  - Row sums of exp(scores) come for free from a ones-column appended to V
    (M=65 stationary); normalization uses a K=1 broadcast matmul + DVE
    reciprocal/multiply.
"""

import os
import sys

for _p in ("/opt/trn_rl_repo",):
    if os.path.isdir(_p) and _p not in sys.path:
        sys.path.append(_p)

import ml_dtypes
import numpy as np

import concourse.bass as bass
import concourse.tile as tile
from concourse import bacc, mybir
from concourse.bass_utils import run_bass_kernel_spmd

BF16 = mybir.dt.bfloat16
F16 = mybir.dt.float16
F32 = mybir.dt.float32

B, S, DM = 4, 2048, 512
H, DH = 8, 64
HPC = 4  # heads per core
DQ = HPC * DH  # 256: per-core slice of the inner dim
N_CORES = 8
SCALE = DH**-0.5

AF = mybir.ActivationFunctionType

# exported for test harnesses
LAST_EXEC_TIME_NS = None
LAST_RESULT = None

_CACHED_NC = None


def _kernel_body(tc, xT_d, wq_d, wk_d, wv_d, wo_d, out_d):
    from contextlib import ExitStack

    nc = tc.nc
    with ExitStack() as ctx:
        consts = ctx.enter_context(tc.tile_pool(name="consts", bufs=1))
        ptp = ctx.enter_context(tc.tile_pool(name="pt", bufs=4))
        normp = ctx.enter_context(tc.tile_pool(name="norm", bufs=2))
        foutp = ctx.enter_context(tc.tile_pool(name="fout", bufs=3))
        # PSUM: "s" 2x[128,1024]=4 banks, "o" 2x[65,512]=2, "b" 1, "x" 1 -> 8
        ps_s = ctx.enter_context(tc.tile_pool(name="ps_s", bufs=2, space="PSUM"))
        ps_o = ctx.enter_context(tc.tile_pool(name="ps_o", bufs=3, space="PSUM"))
        ps_x = ctx.enter_context(tc.tile_pool(name="ps_x", bufs=1, space="PSUM"))
        drp = ctx.enter_context(tc.tile_pool(name="dram", bufs=2, space="DRAM"))

        sb_xT = consts.tile([128, 4, S], BF16)  # X^T: k-chunk c -> [:, c, :]
        sb_wq = consts.tile([128, 4, DQ], BF16)
        sb_wk = consts.tile([128, 4, DQ], BF16)
        sb_wv = consts.tile([128, 4, DQ], BF16)
        sb_wo = consts.tile([128, 2, DM], BF16)  # d'-chunk p -> [:, p, :]
        sb_qT = consts.tile([128, 2, S], BF16)  # dq-chunk (head pair) p
        sb_kT = consts.tile([128, 2, S], BF16)
        sb_v = consts.tile([128, 16, HPC, 66], BF16)  # V_aug; col 64 = ones
        sb_oT = consts.tile([128, 2, S], BF16)  # normalized O^T
        sb_warm = consts.tile([128, 512], BF16)  # PE warmup fodder
        sb_one = consts.tile([128, 64], F16)  # all-ones (bcast stationary)

        nc.vector.memset(sb_one[:], 1.0)
        nc.vector.memset(sb_v[:, :, :, 64:66], 1.0)
        nc.vector.memset(sb_warm[:], 1.0)
        for w_d, w_sb in ((wq_d, sb_wq), (wk_d, sb_wk), (wv_d, sb_wv)):
            nc.sync.dma_start(w_sb[:], w_d.rearrange("(c p) d -> p c d", p=128))
        nc.sync.dma_start(sb_wo[:], wo_d.rearrange("(c p) d -> p c d", p=128))
        xT_r = xT_d.rearrange("(c p) s -> c p s", p=128)
        for kc in range(4):
            nc.sync.dma_start(sb_xT[:, kc, :], xT_r[kc])

        # Warm the PE (HAM un-throttle needs ~3.4us of sustained matmul) and
        # preload the exp table while the xT DMA is in flight. Enough dummy
        # matmuls to keep PE busy until the DMA lands (else the MID window
        # re-throttles it right before the real work starts).
        pw = ps_x.tile([128, 512], F32, tag="x")
        for r in range(24):
            nc.tensor.matmul(
                pw[:], lhsT=sb_warm[:, 0:128], rhs=sb_warm[:], start=True, stop=True
            )
        warm_act = normp.tile([1, 4], F32, tag="wact")
        nc.scalar.activation(warm_act[:], pw[0:1, 0:4], AF.Exp, scale=-1.0)

        def emit_qk_chunk(w_sb, dst_sb, p, c):
            """One [128,512] chunk of Q^T or K^T for head-pair p."""
            isl = slice(c * 512, (c + 1) * 512)
            pq = ps_s.tile([128, 512], F32, tag="s", name="pqk")
            for kc in range(4):
                nc.tensor.matmul(
                    pq[:],
                    lhsT=w_sb[:, kc, p * 128 : (p + 1) * 128],
                    rhs=sb_xT[:, kc, isl],
                    start=(kc == 0),
                    stop=(kc == 3),
                )
            nc.vector.tensor_copy(dst_sb[:, p, isl], pq[:])

        def emit_qk_chunk_mm(w_sb, p, c, kc, pq):
            nc.tensor.matmul(
                pq[:],
                lhsT=w_sb[:, kc, p * 128 : (p + 1) * 128],
                rhs=sb_xT[:, kc, c * 512 : (c + 1) * 512],
                start=(kc == 0),
                stop=(kc == 3),
            )

        def emit_v_chunk(sc):
            """V natural [s,dv] for s-chunk sc (all 4 heads)."""
            pv = ps_x.tile([128, DQ], F32, tag="x", name="pv")
            for kc in range(4):
                nc.tensor.matmul(
                    pv[:],
                    lhsT=sb_xT[:, kc, sc * 128 : (sc + 1) * 128],
                    rhs=sb_wv[:, kc, :],
                    start=(kc == 0),
                    stop=(kc == 3),
                )
            nc.vector.tensor_copy(
                sb_v[:, sc, :, 0:64], pv.rearrange("p (h d) -> p h d", h=HPC)
            )

        def emit_proj_chunk(c2, tag="x"):
            pf = ps_x.tile([128, 512], F32, tag=tag, name="pf") if tag == "x" else (
                ps_o.tile([128, 512], F32, tag=tag, name="pf2")
            )
            for p in range(2):
                nc.tensor.matmul(
                    pf[:],
                    lhsT=sb_oT[:, p, c2 * 128 : (c2 + 1) * 128],
                    rhs=sb_wo[:, p, :],
                    start=(p == 0),
                    stop=(p == 1),
                )
            fo = foutp.tile([128, 512], F32, tag="fo")
            nc.vector.tensor_copy(fo[:], pf[:])
            nc.sync.dma_start(out_d[c2 * 128 : (c2 + 1) * 128, :], fo[:])

        # ---- lead: Q^T/K^T for pair 0, first two V chunks ----
        for c in range(4):
            emit_qk_chunk(sb_wq, sb_qT, 0, c)
        for c in range(4):
            emit_qk_chunk(sb_wk, sb_kT, 0, c)
        emit_v_chunk(0)
        emit_v_chunk(1)

        # deferred work interleaved into attention blocks, one MM per j-iter
        pending_qk = []  # (w_sb, dst_sb, p, c) flattened to per-MM granularity
        for c in range(4):
            pending_qk.append((sb_wk, sb_kT, 1, c))
        for c in range(4):
            pending_qk.append((sb_wq, sb_qT, 1, c))
        qk_state = {"chunk": None, "tile": None, "kc": 0}

        def step_pending_qk():
            stt = qk_state
            if stt["chunk"] is None:
                if not pending_qk:
                    return
                stt["chunk"] = pending_qk.pop(0)
                stt["tile"] = ps_x.tile([128, 512], F32, tag="x", name="pqk1")
                stt["kc"] = 0
            w_sb, dst_sb, p, c = stt["chunk"]
            emit_qk_chunk_mm(w_sb, p, c, stt["kc"], stt["tile"])
            stt["kc"] += 1
            if stt["kc"] == 4:
                nc.vector.tensor_copy(
                    dst_sb[:, p, c * 512 : (c + 1) * 512], stt["tile"][:]
                )
                stt["chunk"] = None

        # ---- attention: pair 0 then pair 1 ----
        # Normalization of block k is emitted lazily, interleaved into the
        # first iterations of block k+1, so the in-order PE stream never
        # stalls long enough for HAM to re-throttle the clock.
        def emit_normalize(p, ic, po):
            """Normalize block (p, ic): 1/rowsum via DVE reciprocal + DMA
            stride-0 broadcast + DVE multiply. No PE instructions at all, so
            the in-order PE stream flows straight into the next block."""
            isl = slice(ic * 512, (ic + 1) * 512)
            for hi in range(2):
                # Row-64 exp-sums -> fp16 (0.05% err) -> K=1 broadcast matmul
                # (fp16 runs at full PE speed; fp32 here split into slow
                # sub-instructions and stalled the whole boundary) -> f32
                # reciprocal at base partition 0 (custom-DVE needs base 0).
                sums = normp.tile([65, 512], F16, tag="sums", name=f"sums{hi}")
                nc.vector.tensor_copy(sums[64:65, :], po[hi][64:65, :])
                pb = ps_o.tile([64, 512], F32, tag="o", name=f"pb{hi}")
                nc.tensor.matmul(
                    pb[:],
                    lhsT=sb_one[64:65, :],
                    rhs=sums[64:65, :],
                    start=True,
                    stop=True,
                )
                rec = normp.tile([64, 512], F32, tag="rec", name=f"rec{hi}")
                nc.vector.reciprocal_approx_fast(rec[:], pb[:])
                if hi == 0:
                    nc.vector.tensor_mul(
                        sb_oT[0:64, p, isl], po[0][0:64, :], rec[:]
                    )
                else:
                    tmpb = normp.tile([64, 512], BF16, tag="tmpb")
                    nc.vector.tensor_mul(tmpb[:], po[1][0:64, :], rec[:])
                    nc.sync.dma_start(sb_oT[64:128, p, isl], tmpb[:])

        # per-MM-granularity deferred projection chunks (run during p1 blocks)
        pending_proj = []
        proj_state = {"c2": None, "tile": None, "p": 0}

        def step_pending_proj():
            stt = proj_state
            if stt["c2"] is None:
                if not pending_proj:
                    return
                stt["c2"] = pending_proj.pop(0)
                stt["tile"] = ps_x.tile([128, 512], F32, tag="x", name="pf")
                stt["p"] = 0
            c2, p = stt["c2"], stt["p"]
            nc.tensor.matmul(
                stt["tile"][:],
                lhsT=sb_oT[:, p, c2 * 128 : (c2 + 1) * 128],
                rhs=sb_wo[:, p, :],
                start=(p == 0),
                stop=(p == 1),
            )
            stt["p"] += 1
            if stt["p"] == 2:
                fo = foutp.tile([128, 512], F32, tag="fo")
                nc.vector.tensor_copy(fo[:], stt["tile"][:])
                nc.sync.dma_start(out_d[c2 * 128 : (c2 + 1) * 128, :], fo[:])
                stt["c2"] = None

        for p in range(2):
            for ic in range(4):
                isl = slice(ic * 512, (ic + 1) * 512)
                po = [
                    ps_o.tile([65, 512], F32, tag="o", name=f"po{hi}")
                    for hi in range(2)
                ]
                for j in range(16):
                    jsl = slice(j * 128, (j + 1) * 128)
                    # extras: one deferred matmul per iteration keeps PE fed
                    # without outrunning ACT
                    if p == 0 and ic == 0:
                        if j < 14:
                            emit_v_chunk(j + 2)
                    elif p == 0:
                        step_pending_qk()
                    else:
                        step_pending_proj()
                    st = ps_s.tile([128, 1024], F32, tag="s")
                    # two K=64 row-tiles run concurrently in the PE array
                    nc.tensor.matmul(
                        st[:, 0:512],
                        lhsT=sb_kT[0:64, p, jsl],
                        rhs=sb_qT[0:64, p, isl],
                        start=True,
                        stop=True,
                    )
                    nc.tensor.matmul(
                        st[:, 512:1024],
                        lhsT=sb_kT[64:128, p, jsl],
                        rhs=sb_qT[64:128, p, isl],
                        start=True,
                        stop=True,
                    )
                    pt = ptp.tile([128, 1024], BF16, tag="pt")
                    nc.scalar.activation(pt[:], st[:], AF.Exp, scale=SCALE)
                    for hi in range(2):
                        nc.tensor.matmul(
                            po[hi][:],
                            lhsT=sb_v[:, j, 2 * p + hi, 0:65],
                            rhs=pt[:, hi * 512 : (hi + 1) * 512],
                            start=(j == 0),
                            stop=(j == 15),
                            skip_group_check=True,
                        )
                emit_normalize(p, ic, po)
                if p == 1 and ic > 0:
                    pending_proj.extend(range(4 * (ic - 1), 4 * ic))

        # ---- tail: remaining projection chunks ----
        while pending_proj or proj_state["c2"] is not None:
            step_pending_proj()
        for c2 in range(12, 16):
            emit_proj_chunk(c2, tag="o" if c2 % 2 else "x")


def _build():
    nc = bacc.Bacc("TRN2", target_bir_lowering=False, debug=False, num_devices=N_CORES)
    xT = nc.dram_tensor("xT", [DM, S], BF16, kind="ExternalInput")
    wq = nc.dram_tensor("wq", [DM, DQ], BF16, kind="ExternalInput")
    wk = nc.dram_tensor("wk", [DM, DQ], BF16, kind="ExternalInput")
    wv = nc.dram_tensor("wv", [DM, DQ], BF16, kind="ExternalInput")
    wo = nc.dram_tensor("wo", [DQ, DM], BF16, kind="ExternalInput")
    out = nc.dram_tensor("out", [S, DM], F32, kind="ExternalOutput")
    with tile.TileContext(nc) as tc:
        _kernel_body(tc, xT.ap(), wq.ap(), wk.ap(), wv.ap(), wo.ap(), out.ap())
    nc.compile()
    return nc


def get_nc():
    global _CACHED_NC
    if _CACHED_NC is None:
        _CACHED_NC = _build()
    return _CACHED_NC


def _in_maps(hidden_states, Wq, Wk, Wv, Wo):
    bf = ml_dtypes.bfloat16
    maps = []
    for c in range(N_CORES):
        b, g = c // 2, c % 2
        cols = slice(g * DQ, (g + 1) * DQ)
        maps.append(
            {
                "xT": np.ascontiguousarray(hidden_states[b].T).astype(bf),
                "wq": np.ascontiguousarray(Wq[:, cols]).astype(bf),
                "wk": np.ascontiguousarray(Wk[:, cols]).astype(bf),
                "wv": np.ascontiguousarray(Wv[:, cols]).astype(bf),
                "wo": np.ascontiguousarray(Wo[cols, :]).astype(bf),
            }
        )
    return maps


def _ensure_profile_support():
    """Best-effort: register the axon NTFF profiling hook + defang the
    bucket upload (zero-egress container). Without this, trace=True dies
    on a missing ``antenv.axon_hooks`` module in this image."""
    import types

    try:
        import antenv

        if "antenv.axon_hooks" not in sys.modules:
            mod = types.ModuleType("antenv.axon_hooks")
            _h = {"hook": None}
            mod.set_axon_ntff_profile_hook = lambda h: _h.__setitem__("hook", h)
            mod.get_axon_ntff_profile_hook = lambda: _h["hook"]
            sys.modules["antenv.axon_hooks"] = mod
            antenv.axon_hooks = mod
        import antenv.axon_hooks as ah

        if ah.get_axon_ntff_profile_hook() is None:
            if "/root/.axon_site" not in sys.path:
                sys.path.append("/root/.axon_site")
            from trn_agent_boot.trn_boot import _ntff_profile_via_ctypes

            hook = _ntff_profile_via_ctypes("/opt/axon/libaxon_pjrt.so")
            if hook is not None:
                ah.set_axon_ntff_profile_hook(hook)
    except Exception:
        pass
    try:
        import concourse.bass_utils as bu

        bu.upload_artifacts = lambda tmpdir: tmpdir
    except Exception:
        pass


def kernel(hidden_states, Wq, Wk, Wv, Wo):
    global LAST_EXEC_TIME_NS, LAST_RESULT
    hidden_states = np.asarray(hidden_states, dtype=np.float32)
    Wq, Wk, Wv, Wo = (np.asarray(w, dtype=np.float32) for w in (Wq, Wk, Wv, Wo))

    trace = bool(os.environ.get("BASS_TRACE"))
    if trace:
        _ensure_profile_support()
    nc = get_nc()
    maps = _in_maps(hidden_states, Wq, Wk, Wv, Wo)
    res = run_bass_kernel_spmd(
        nc,
        maps,
        core_ids=list(range(N_CORES)),
        trace=trace,
        tmpdir=os.environ.get("BASS_TRACE_DIR") or None,
    )
    LAST_RESULT = res
    LAST_EXEC_TIME_NS = res.exec_time_ns

    out = np.empty((B, S, DM), dtype=np.float32)
    for b in range(B):
        out[b] = res.results[2 * b]["out"] + res.results[2 * b + 1]["out"]
    return out


if __name__ == "__main__":
    rng = np.random.default_rng(0)
    hs = rng.standard_normal((B, S, DM), dtype=np.float32)
    ws = [
        (rng.standard_normal((DM, DM), dtype=np.float32) / np.sqrt(DM))
        for _ in range(4)
    ]
    o = kernel(hs, *ws)
    print("out", o.shape, o.dtype, float(np.abs(o).mean()))
    print("exec_time_ns", LAST_EXEC_TIME_NS)


# revision 20
# speedup vs baseline: 1.1085x; 1.0061x over previous
"""Multi-head attention (B=4, S=2048, H=8, Dh=64, Dm=512) on 8 TRN2 NeuronCores.

Sharding: batch*head parallel. Core c owns batch b = c//2 and head group
g = c%2 (4 heads each). Each core computes QKV projection for its head
group, transposed-scores flash-style attention (no max subtraction --
scores ~ N(0,1) after 1/sqrt(Dh) scaling, exp is safe in fp32/bf16), and
its partial output projection against its 256 rows of Wo. The host sums
the two partial projections per batch.

Device-side layout notes:
  - X^T (bf16) is prepared on host so every matmul contracts over the
    partition dim directly.
  - Scores are computed transposed (S^T[j,i] = K Q^T) so the attention*V
    matmul needs no transposition; the two heads of a 128-row Q^T/K^T
    chunk are packed into the PE array as two K=64 row-tiles (tile_position
    (0,0)/(64,0)) running concurrently.
  - Row sums of exp(scores) come for free from a ones-column appended to V
    (M=65 stationary); normalization uses a K=1 broadcast matmul + DVE
    reciprocal/multiply.
"""

import os
import sys

for _p in ("/opt/trn_rl_repo",):
    if os.path.isdir(_p) and _p not in sys.path:
        sys.path.append(_p)

import ml_dtypes
import numpy as np

import concourse.bass as bass
import concourse.tile as tile
from concourse import bacc, mybir
from concourse.bass_utils import run_bass_kernel_spmd

BF16 = mybir.dt.bfloat16
F16 = mybir.dt.float16
F32 = mybir.dt.float32

B, S, DM = 4, 2048, 512
H, DH = 8, 64
HPC = 4  # heads per core
DQ = HPC * DH  # 256: per-core slice of the inner dim
N_CORES = 8
SCALE = DH**-0.5

AF = mybir.ActivationFunctionType

# exported for test harnesses
LAST_EXEC_TIME_NS = None
LAST_RESULT = None

_CACHED_NC = None


def _kernel_body(tc, xT_d, wq_d, wk_d, wv_d, wo_d, out_d):
    from contextlib import ExitStack

    nc = tc.nc
    with ExitStack() as ctx:
        consts = ctx.enter_context(tc.tile_pool(name="consts", bufs=1))
        ptp = ctx.enter_context(tc.tile_pool(name="pt", bufs=4))
        normp = ctx.enter_context(tc.tile_pool(name="norm", bufs=2))
        foutp = ctx.enter_context(tc.tile_pool(name="fout", bufs=3))
        # PSUM: "s" 2x[128,1024]=4 banks, "o" 2x[65,512]=2, "b" 1, "x" 1 -> 8
        ps_s = ctx.enter_context(tc.tile_pool(name="ps_s", bufs=2, space="PSUM"))
        ps_o = ctx.enter_context(tc.tile_pool(name="ps_o", bufs=3, space="PSUM"))
        ps_x = ctx.enter_context(tc.tile_pool(name="ps_x", bufs=1, space="PSUM"))
        drp = ctx.enter_context(tc.tile_pool(name="dram", bufs=2, space="DRAM"))

        sb_xT = consts.tile([128, 4, S], BF16)  # X^T: k-chunk c -> [:, c, :]
        sb_wq = consts.tile([128, 4, DQ], BF16)
        sb_wk = consts.tile([128, 4, DQ], BF16)
        sb_wv = consts.tile([128, 4, DQ], BF16)
        sb_wo = consts.tile([128, 2, DM], BF16)  # d'-chunk p -> [:, p, :]
        sb_qT = consts.tile([128, 2, S], BF16)  # dq-chunk (head pair) p
        sb_kT = consts.tile([128, 2, S], BF16)
        sb_v = consts.tile([128, 16, HPC, 66], BF16)  # V_aug; col 64 = ones
        sb_oT = consts.tile([128, 2, S], BF16)  # normalized O^T
        sb_warm = consts.tile([128, 512], BF16)  # PE warmup fodder
        sb_one = consts.tile([128, 64], F16)  # all-ones (bcast stationary)

        nc.vector.memset(sb_one[:], 1.0)
        nc.vector.memset(sb_v[:, :, :, 64:66], 1.0)
        nc.vector.memset(sb_warm[:], 1.0)
        for w_d, w_sb in ((wq_d, sb_wq), (wk_d, sb_wk), (wv_d, sb_wv)):
            nc.sync.dma_start(w_sb[:], w_d.rearrange("(c p) d -> p c d", p=128))
        nc.sync.dma_start(sb_wo[:], wo_d.rearrange("(c p) d -> p c d", p=128))
        xT_r = xT_d.rearrange("(c p) s -> c p s", p=128)
        for kc in range(4):
            nc.sync.dma_start(sb_xT[:, kc, :], xT_r[kc])

        # Warm the PE (HAM un-throttle needs ~3.4us of sustained matmul) and
        # preload the exp table while the xT DMA is in flight. Enough dummy
        # matmuls to keep PE busy until the DMA lands (else the MID window
        # re-throttles it right before the real work starts).
        pw = ps_x.tile([128, 512], F32, tag="x")
        for r in range(24):
            nc.tensor.matmul(
                pw[:], lhsT=sb_warm[:, 0:128], rhs=sb_warm[:], start=True, stop=True
            )
        warm_act = normp.tile([1, 4], F32, tag="wact")
        nc.scalar.activation(warm_act[:], pw[0:1, 0:4], AF.Exp, scale=-1.0)

        def emit_qk_chunk(w_sb, dst_sb, p, c):
            """One [128,512] chunk of Q^T or K^T for head-pair p."""
            isl = slice(c * 512, (c + 1) * 512)
            pq = ps_s.tile([128, 512], F32, tag="s", name="pqk")
            for kc in range(4):
                nc.tensor.matmul(
                    pq[:],
                    lhsT=w_sb[:, kc, p * 128 : (p + 1) * 128],
                    rhs=sb_xT[:, kc, isl],
                    start=(kc == 0),
                    stop=(kc == 3),
                )
            nc.vector.tensor_copy(dst_sb[:, p, isl], pq[:])

        def emit_qk_chunk_mm(w_sb, p, c, kc, pq):
            nc.tensor.matmul(
                pq[:],
                lhsT=w_sb[:, kc, p * 128 : (p + 1) * 128],
                rhs=sb_xT[:, kc, c * 512 : (c + 1) * 512],
                start=(kc == 0),
                stop=(kc == 3),
            )

        def emit_v_chunk(sc):
            """V natural [s,dv] for s-chunk sc (all 4 heads)."""
            pv = ps_x.tile([128, DQ], F32, tag="x", name="pv")
            for kc in range(4):
                nc.tensor.matmul(
                    pv[:],
                    lhsT=sb_xT[:, kc, sc * 128 : (sc + 1) * 128],
                    rhs=sb_wv[:, kc, :],
                    start=(kc == 0),
                    stop=(kc == 3),
                )
            nc.vector.tensor_copy(
                sb_v[:, sc, :, 0:64], pv.rearrange("p (h d) -> p h d", h=HPC)
            )

        def emit_proj_chunk(c2, tag="x"):
            pf = ps_x.tile([128, 512], F32, tag=tag, name="pf") if tag == "x" else (
                ps_o.tile([128, 512], F32, tag=tag, name="pf2")
            )
            for p in range(2):
                nc.tensor.matmul(
                    pf[:],
                    lhsT=sb_oT[:, p, c2 * 128 : (c2 + 1) * 128],
                    rhs=sb_wo[:, p, :],
                    start=(p == 0),
                    stop=(p == 1),
                )
            fo = foutp.tile([128, 512], F32, tag="fo")
            nc.vector.tensor_copy(fo[:], pf[:])
            nc.sync.dma_start(out_d[c2 * 128 : (c2 + 1) * 128, :], fo[:])

        # ---- lead: Q^T (all chunks) for pair 0 + first K^T chunk; the
        # remaining K^T chunks and all V chunks stream inside block 0 ----
        for c in range(4):
            emit_qk_chunk(sb_wq, sb_qT, 0, c)
        emit_qk_chunk(sb_wk, sb_kT, 0, 0)

        # deferred work interleaved into attention blocks, one MM per j-iter
        pending_qk = []  # (w_sb, dst_sb, p, c) flattened to per-MM granularity
        for c in range(4):
            pending_qk.append((sb_wk, sb_kT, 1, c))
        for c in range(4):
            pending_qk.append((sb_wq, sb_qT, 1, c))
        qk_state = {"chunk": None, "tile": None, "kc": 0}

        def step_pending_qk():
            stt = qk_state
            if stt["chunk"] is None:
                if not pending_qk:
                    return
                stt["chunk"] = pending_qk.pop(0)
                stt["tile"] = ps_x.tile([128, 512], F32, tag="x", name="pqk1")
                stt["kc"] = 0
            w_sb, dst_sb, p, c = stt["chunk"]
            emit_qk_chunk_mm(w_sb, p, c, stt["kc"], stt["tile"])
            stt["kc"] += 1
            if stt["kc"] == 4:
                nc.vector.tensor_copy(
                    dst_sb[:, p, c * 512 : (c + 1) * 512], stt["tile"][:]
                )
                stt["chunk"] = None

        # ---- attention: pair 0 then pair 1 ----
        # Normalization of block k is emitted lazily, interleaved into the
        # first iterations of block k+1, so the in-order PE stream never
        # stalls long enough for HAM to re-throttle the clock.
        def make_norm_steps(p, ic, po):
            """Normalization of a finished block, split into 3 steps that the
            next block interleaves into its first iterations (the fp16 K=1
            broadcast matmuls sit behind fresh scores in PE order, so the PE
            never stalls waiting on the DVE sums copies)."""
            isl = slice(ic * 512, (ic + 1) * 512)
            held = {}

            def step_sums():
                for hi in range(2):
                    s = normp.tile([65, 512], F16, tag="sums", name=f"sums{hi}")
                    nc.vector.tensor_copy(s[64:65, :], po[hi][64:65, :])
                    held[hi] = s

            def step_head(hi):
                pb = ps_o.tile([64, 512], F32, tag="o", name=f"pb{hi}")
                nc.tensor.matmul(
                    pb[:],
                    lhsT=sb_one[64:65, :],
                    rhs=held[hi][64:65, :],
                    start=True,
                    stop=True,
                )
                rec = normp.tile([64, 512], F32, tag="rec", name=f"rec{hi}")
                nc.vector.reciprocal_approx_fast(rec[:], pb[:])
                if hi == 0:
                    nc.vector.tensor_mul(
                        sb_oT[0:64, p, isl], po[0][0:64, :], rec[:]
                    )
                else:
                    tmpb = normp.tile([64, 512], BF16, tag="tmpb")
                    nc.vector.tensor_mul(tmpb[:], po[1][0:64, :], rec[:])
                    nc.sync.dma_start(sb_oT[64:128, p, isl], tmpb[:])

            return [step_sums, lambda: step_head(0), lambda: step_head(1)]

        # per-MM-granularity deferred projection chunks (run during p1 blocks)
        pending_proj = []
        proj_state = {"c2": None, "tile": None, "p": 0}

        def step_pending_proj():
            stt = proj_state
            if stt["c2"] is None:
                if not pending_proj:
                    return
                stt["c2"] = pending_proj.pop(0)
                stt["tile"] = ps_x.tile([128, 512], F32, tag="x", name="pf")
                stt["p"] = 0
            c2, p = stt["c2"], stt["p"]
            nc.tensor.matmul(
                stt["tile"][:],
                lhsT=sb_oT[:, p, c2 * 128 : (c2 + 1) * 128],
                rhs=sb_wo[:, p, :],
                start=(p == 0),
                stop=(p == 1),
            )
            stt["p"] += 1
            if stt["p"] == 2:
                fo = foutp.tile([128, 512], F32, tag="fo")
                nc.vector.tensor_copy(fo[:], stt["tile"][:])
                nc.sync.dma_start(out_d[c2 * 128 : (c2 + 1) * 128, :], fo[:])
                stt["c2"] = None

        pending_norm = []
        for p in range(2):
            for ic in range(4):
                isl = slice(ic * 512, (ic + 1) * 512)
                po = [
                    ps_o.tile([65, 512], F32, tag="o", name=f"po{hi}")
                    for hi in range(2)
                ]
                for j in range(16):
                    jsl = slice(j * 128, (j + 1) * 128)
                    if j == 0 and pending_norm:
                        pending_norm[0]()  # sums copies (DVE only)
                    st = ps_s.tile([128, 1024], F32, tag="s")
                    # two K=64 row-tiles run concurrently in the PE array
                    nc.tensor.matmul(
                        st[:, 0:512],
                        lhsT=sb_kT[0:64, p, jsl],
                        rhs=sb_qT[0:64, p, isl],
                        start=True,
                        stop=True,
                    )
                    nc.tensor.matmul(
                        st[:, 512:1024],
                        lhsT=sb_kT[64:128, p, jsl],
                        rhs=sb_qT[64:128, p, isl],
                        start=True,
                        stop=True,
                    )
                    if j == 0 and pending_norm:
                        pending_norm[1]()  # bcast+recip+mul head 0
                        pending_norm[2]()  # bcast+recip+mul head 1
                        pending_norm = []
                    # extras: deferred matmuls keep PE fed without a long
                    # serial lead; emitted after scores so ACT starts sooner
                    if p == 0 and ic == 0:
                        if 0 < j < 4:
                            emit_qk_chunk(sb_wk, sb_kT, 0, j)
                        if j == 0:
                            emit_v_chunk(0)
                            emit_v_chunk(1)
                        elif j < 15:
                            emit_v_chunk(j + 1)
                    elif p == 0:
                        step_pending_qk()
                    else:
                        step_pending_proj()
                    pt = ptp.tile([128, 1024], BF16, tag="pt")
                    nc.scalar.activation(pt[:], st[:], AF.Exp, scale=SCALE)
                    for hi in range(2):
                        nc.tensor.matmul(
                            po[hi][:],
                            lhsT=sb_v[:, j, 2 * p + hi, 0:65],
                            rhs=pt[:, hi * 512 : (hi + 1) * 512],
                            start=(j == 0),
                            stop=(j == 15),
                            skip_group_check=True,
                        )
                pending_norm = make_norm_steps(p, ic, po)
                if p == 1 and ic > 0:
                    pending_proj.extend(range(4 * (ic - 1), 4 * ic))

        # ---- tail: last normalize + remaining projection chunks ----
        for step in pending_norm:
            step()
        while pending_proj or proj_state["c2"] is not None:
            step_pending_proj()
        for c2 in range(12, 16):
            emit_proj_chunk(c2, tag="o" if c2 % 2 else "x")


def _build():
    nc = bacc.Bacc("TRN2", target_bir_lowering=False, debug=False, num_devices=N_CORES)
    xT = nc.dram_tensor("xT", [DM, S], BF16, kind="ExternalInput")
    wq = nc.dram_tensor("wq", [DM, DQ], BF16, kind="ExternalInput")
    wk = nc.dram_tensor("wk", [DM, DQ], BF16, kind="ExternalInput")
    wv = nc.dram_tensor("wv", [DM, DQ], BF16, kind="ExternalInput")
    wo = nc.dram_tensor("wo", [DQ, DM], BF16, kind="ExternalInput")
    out = nc.dram_tensor("out", [S, DM], F32, kind="ExternalOutput")
    with tile.TileContext(nc) as tc:
        _kernel_body(tc, xT.ap(), wq.ap(), wk.ap(), wv.ap(), wo.ap(), out.ap())
    nc.compile()
    return nc


def get_nc():
    global _CACHED_NC
    if _CACHED_NC is None:
        _CACHED_NC = _build()
    return _CACHED_NC


def _in_maps(hidden_states, Wq, Wk, Wv, Wo):
    bf = ml_dtypes.bfloat16
    maps = []
    for c in range(N_CORES):
        b, g = c // 2, c % 2
        cols = slice(g * DQ, (g + 1) * DQ)
        maps.append(
            {
                "xT": np.ascontiguousarray(hidden_states[b].T).astype(bf),
                "wq": np.ascontiguousarray(Wq[:, cols]).astype(bf),
                "wk": np.ascontiguousarray(Wk[:, cols]).astype(bf),
                "wv": np.ascontiguousarray(Wv[:, cols]).astype(bf),
                "wo": np.ascontiguousarray(Wo[cols, :]).astype(bf),
            }
        )
    return maps


def _ensure_profile_support():
    """Best-effort: register the axon NTFF profiling hook + defang the
    bucket upload (zero-egress container). Without this, trace=True dies
    on a missing ``antenv.axon_hooks`` module in this image."""
    import types

    try:
        import antenv

        if "antenv.axon_hooks" not in sys.modules:
            mod = types.ModuleType("antenv.axon_hooks")
            _h = {"hook": None}
            mod.set_axon_ntff_profile_hook = lambda h: _h.__setitem__("hook", h)
            mod.get_axon_ntff_profile_hook = lambda: _h["hook"]
            sys.modules["antenv.axon_hooks"] = mod
            antenv.axon_hooks = mod
        import antenv.axon_hooks as ah

        if ah.get_axon_ntff_profile_hook() is None:
            if "/root/.axon_site" not in sys.path:
                sys.path.append("/root/.axon_site")
            from trn_agent_boot.trn_boot import _ntff_profile_via_ctypes

            hook = _ntff_profile_via_ctypes("/opt/axon/libaxon_pjrt.so")
            if hook is not None:
                ah.set_axon_ntff_profile_hook(hook)
    except Exception:
        pass
    try:
        import concourse.bass_utils as bu

        bu.upload_artifacts = lambda tmpdir: tmpdir
    except Exception:
        pass


def kernel(hidden_states, Wq, Wk, Wv, Wo):
    global LAST_EXEC_TIME_NS, LAST_RESULT
    hidden_states = np.asarray(hidden_states, dtype=np.float32)
    Wq, Wk, Wv, Wo = (np.asarray(w, dtype=np.float32) for w in (Wq, Wk, Wv, Wo))

    trace = bool(os.environ.get("BASS_TRACE"))
    if trace:
        _ensure_profile_support()
    nc = get_nc()
    maps = _in_maps(hidden_states, Wq, Wk, Wv, Wo)
    res = run_bass_kernel_spmd(
        nc,
        maps,
        core_ids=list(range(N_CORES)),
        trace=trace,
        tmpdir=os.environ.get("BASS_TRACE_DIR") or None,
    )
    LAST_RESULT = res
    LAST_EXEC_TIME_NS = res.exec_time_ns

    out = np.empty((B, S, DM), dtype=np.float32)
    for b in range(B):
        out[b] = res.results[2 * b]["out"] + res.results[2 * b + 1]["out"]
    return out


if __name__ == "__main__":
    rng = np.random.default_rng(0)
    hs = rng.standard_normal((B, S, DM), dtype=np.float32)
    ws = [
        (rng.standard_normal((DM, DM), dtype=np.float32) / np.sqrt(DM))
        for _ in range(4)
    ]
    o = kernel(hs, *ws)
    print("out", o.shape, o.dtype, float(np.abs(o).mean()))
    print("exec_time_ns", LAST_EXEC_TIME_NS)


# revision 22
# speedup vs baseline: 1.1187x; 1.0092x over previous
"""Multi-head attention (B=4, S=2048, H=8, Dh=64, Dm=512) on 8 TRN2 NeuronCores.

Sharding: batch*head parallel. Core c owns batch b = c//2 and head group
g = c%2 (4 heads each). Each core computes QKV projection for its head
group, transposed-scores flash-style attention (no max subtraction --
scores ~ N(0,1) after 1/sqrt(Dh) scaling, exp is safe in fp32/bf16), and
its partial output projection against its 256 rows of Wo. The host sums
the two partial projections per batch.

Device-side layout notes:
  - X^T (bf16) is prepared on host so every matmul contracts over the
    partition dim directly.
  - Scores are computed transposed (S^T[j,i] = K Q^T) so the attention*V
    matmul needs no transposition; the two heads of a 128-row Q^T/K^T
    chunk are packed into the PE array as two K=64 row-tiles (tile_position
    (0,0)/(64,0)) running concurrently.
  - Row sums of exp(scores) come for free from a ones-column appended to V
    (M=65 stationary); normalization uses a K=1 broadcast matmul + DVE
    reciprocal/multiply.
"""

import os
import sys

for _p in ("/opt/trn_rl_repo",):
    if os.path.isdir(_p) and _p not in sys.path:
        sys.path.append(_p)

import ml_dtypes
import numpy as np

import concourse.bass as bass
import concourse.tile as tile
from concourse import bacc, mybir
from concourse.bass_utils import run_bass_kernel_spmd

BF16 = mybir.dt.bfloat16
F16 = mybir.dt.float16
F32 = mybir.dt.float32

B, S, DM = 4, 2048, 512
H, DH = 8, 64
HPC = 4  # heads per core
DQ = HPC * DH  # 256: per-core slice of the inner dim
N_CORES = 8
SCALE = DH**-0.5

AF = mybir.ActivationFunctionType

# exported for test harnesses
LAST_EXEC_TIME_NS = None
LAST_RESULT = None

_CACHED_NC = None


def _kernel_body(tc, xT_d, wq_d, wk_d, wv_d, wo_d, out_d):
    from contextlib import ExitStack

    nc = tc.nc
    with ExitStack() as ctx:
        consts = ctx.enter_context(tc.tile_pool(name="consts", bufs=1))
        ptp = ctx.enter_context(tc.tile_pool(name="pt", bufs=6))
        normp = ctx.enter_context(tc.tile_pool(name="norm", bufs=2))
        foutp = ctx.enter_context(tc.tile_pool(name="fout", bufs=3))
        # PSUM: "s" 2x[128,1024]=4 banks, "o" 2x[65,512]=2, "b" 1, "x" 1 -> 8
        ps_s = ctx.enter_context(tc.tile_pool(name="ps_s", bufs=2, space="PSUM"))
        ps_o = ctx.enter_context(tc.tile_pool(name="ps_o", bufs=3, space="PSUM"))
        ps_x = ctx.enter_context(tc.tile_pool(name="ps_x", bufs=1, space="PSUM"))
        drp = ctx.enter_context(tc.tile_pool(name="dram", bufs=2, space="DRAM"))

        sb_xT = consts.tile([128, 4, S], BF16)  # X^T: k-chunk c -> [:, c, :]
        sb_wq = consts.tile([128, 4, DQ], BF16)
        sb_wk = consts.tile([128, 4, DQ], BF16)
        sb_wv = consts.tile([128, 4, DQ], BF16)
        sb_wo = consts.tile([128, 2, DM], BF16)  # d'-chunk p -> [:, p, :]
        sb_qT = consts.tile([128, 2, S], BF16)  # dq-chunk (head pair) p
        sb_kT = consts.tile([128, 2, S], BF16)
        sb_v = consts.tile([128, 16, HPC, 66], BF16)  # V_aug; col 64 = ones
        sb_oT = consts.tile([128, 2, S], BF16)  # normalized O^T
        sb_warm = consts.tile([128, 512], BF16)  # PE warmup fodder
        sb_one = consts.tile([128, 64], F16)  # all-ones (bcast stationary)

        nc.vector.memset(sb_one[:], 1.0)
        nc.vector.memset(sb_v[:, :, :, 64:66], 1.0)
        nc.vector.memset(sb_warm[:], 1.0)
        for w_d, w_sb in ((wq_d, sb_wq), (wk_d, sb_wk), (wv_d, sb_wv)):
            nc.sync.dma_start(w_sb[:], w_d.rearrange("(c p) d -> p c d", p=128))
        nc.sync.dma_start(sb_wo[:], wo_d.rearrange("(c p) d -> p c d", p=128))
        xT_r = xT_d.rearrange("(c p) s -> c p s", p=128)
        for kc in range(4):
            nc.sync.dma_start(sb_xT[:, kc, :], xT_r[kc])

        # Warm the PE (HAM un-throttle needs ~3.4us of sustained matmul) and
        # preload the exp table while the xT DMA is in flight. Enough dummy
        # matmuls to keep PE busy until the DMA lands (else the MID window
        # re-throttles it right before the real work starts).
        pw = ps_x.tile([128, 512], F32, tag="x")
        for r in range(24):
            nc.tensor.matmul(
                pw[:], lhsT=sb_warm[:, 0:128], rhs=sb_warm[:], start=True, stop=True
            )
        warm_act = normp.tile([1, 4], F32, tag="wact")
        nc.scalar.activation(warm_act[:], pw[0:1, 0:4], AF.Exp, scale=-1.0)

        def emit_qk_chunk(w_sb, dst_sb, p, c, pool_tag=("ps_s", "s")):
            """One [128,512] chunk of Q^T or K^T for head-pair p."""
            isl = slice(c * 512, (c + 1) * 512)
            pool = {"ps_s": ps_s, "ps_o": ps_o, "ps_x": ps_x}[pool_tag[0]]
            pq = pool.tile([128, 512], F32, tag=pool_tag[1], name="pqk")
            for kc in range(4):
                nc.tensor.matmul(
                    pq[:],
                    lhsT=w_sb[:, kc, p * 128 : (p + 1) * 128],
                    rhs=sb_xT[:, kc, isl],
                    start=(kc == 0),
                    stop=(kc == 3),
                )
            nc.vector.tensor_copy(dst_sb[:, p, isl], pq[:])

        def emit_qk_chunk_mm(w_sb, p, c, kc, pq):
            nc.tensor.matmul(
                pq[:],
                lhsT=w_sb[:, kc, p * 128 : (p + 1) * 128],
                rhs=sb_xT[:, kc, c * 512 : (c + 1) * 512],
                start=(kc == 0),
                stop=(kc == 3),
            )

        def emit_v_chunk(sc):
            """V natural [s,dv] for s-chunk sc (all 4 heads)."""
            pv = ps_x.tile([128, DQ], F32, tag="x", name="pv")
            for kc in range(4):
                nc.tensor.matmul(
                    pv[:],
                    lhsT=sb_xT[:, kc, sc * 128 : (sc + 1) * 128],
                    rhs=sb_wv[:, kc, :],
                    start=(kc == 0),
                    stop=(kc == 3),
                )
            nc.vector.tensor_copy(
                sb_v[:, sc, :, 0:64], pv.rearrange("p (h d) -> p h d", h=HPC)
            )

        def emit_proj_chunk(c2, tag="x"):
            pf = ps_x.tile([128, 512], F32, tag=tag, name="pf") if tag == "x" else (
                ps_o.tile([128, 512], F32, tag=tag, name="pf2")
            )
            for p in range(2):
                nc.tensor.matmul(
                    pf[:],
                    lhsT=sb_oT[:, p, c2 * 128 : (c2 + 1) * 128],
                    rhs=sb_wo[:, p, :],
                    start=(p == 0),
                    stop=(p == 1),
                )
            fo = foutp.tile([128, 512], F32, tag="fo")
            nc.vector.tensor_copy(fo[:], pf[:])
            nc.sync.dma_start(out_d[c2 * 128 : (c2 + 1) * 128, :], fo[:])

        # ---- lead: Q^T (all chunks) for pair 0 + first K^T chunk; the
        # remaining K^T chunks and all V chunks stream inside block 0 ----
        for c in range(4):
            emit_qk_chunk(sb_wq, sb_qT, 0, c)
        emit_qk_chunk(sb_wk, sb_kT, 0, 0)

        # deferred work interleaved into attention blocks, one MM per j-iter
        pending_qk = []  # (w_sb, dst_sb, p, c) flattened to per-MM granularity
        for c in range(4):
            pending_qk.append((sb_wk, sb_kT, 1, c))
        for c in range(4):
            pending_qk.append((sb_wq, sb_qT, 1, c))
        qk_state = {"chunk": None, "tile": None, "kc": 0}

        def step_pending_qk():
            stt = qk_state
            if stt["chunk"] is None:
                if not pending_qk:
                    return
                stt["chunk"] = pending_qk.pop(0)
                stt["tile"] = ps_x.tile([128, 512], F32, tag="x", name="pqk1")
                stt["kc"] = 0
            w_sb, dst_sb, p, c = stt["chunk"]
            emit_qk_chunk_mm(w_sb, p, c, stt["kc"], stt["tile"])
            stt["kc"] += 1
            if stt["kc"] == 4:
                nc.vector.tensor_copy(
                    dst_sb[:, p, c * 512 : (c + 1) * 512], stt["tile"][:]
                )
                stt["chunk"] = None

        # ---- attention: pair 0 then pair 1 ----
        # Normalization of block k is emitted lazily, interleaved into the
        # first iterations of block k+1, so the in-order PE stream never
        # stalls long enough for HAM to re-throttle the clock.
        def make_norm_steps(p, ic, po):
            """Normalization of a finished block, split into 3 steps that the
            next block interleaves into its first iterations (the fp16 K=1
            broadcast matmuls sit behind fresh scores in PE order, so the PE
            never stalls waiting on the DVE sums copies)."""
            isl = slice(ic * 512, (ic + 1) * 512)
            held = {}

            def step_sums():
                for hi in range(2):
                    s = normp.tile([65, 512], F16, tag="sums", name=f"sums{hi}")
                    nc.vector.tensor_copy(s[64:65, :], po[hi][64:65, :])
                    held[hi] = s

            def step_head(hi):
                pb = ps_x.tile([64, 512], F32, tag="x", name=f"pb{hi}")
                nc.tensor.matmul(
                    pb[:],
                    lhsT=sb_one[64:65, :],
                    rhs=held[hi][64:65, :],
                    start=True,
                    stop=True,
                )
                rec = normp.tile([64, 512], F32, tag="rec", name=f"rec{hi}")
                nc.vector.reciprocal_approx_fast(rec[:], pb[:])
                if hi == 0:
                    nc.vector.tensor_mul(
                        sb_oT[0:64, p, isl], po[0][0:64, :], rec[:]
                    )
                else:
                    tmpb = normp.tile([64, 512], BF16, tag="tmpb")
                    nc.vector.tensor_mul(tmpb[:], po[1][0:64, :], rec[:])
                    nc.sync.dma_start(sb_oT[64:128, p, isl], tmpb[:])

            return [step_sums, lambda: step_head(0), lambda: step_head(1)]

        # per-MM-granularity deferred projection chunks (run during p1 blocks)
        pending_proj = []
        proj_state = {"c2": None, "tile": None, "p": 0}

        def step_pending_proj():
            stt = proj_state
            if stt["c2"] is None:
                if not pending_proj:
                    return
                stt["c2"] = pending_proj.pop(0)
                stt["tile"] = ps_x.tile([128, 512], F32, tag="x", name="pf")
                stt["p"] = 0
            c2, p = stt["c2"], stt["p"]
            nc.tensor.matmul(
                stt["tile"][:],
                lhsT=sb_oT[:, p, c2 * 128 : (c2 + 1) * 128],
                rhs=sb_wo[:, p, :],
                start=(p == 0),
                stop=(p == 1),
            )
            stt["p"] += 1
            if stt["p"] == 2:
                fo = foutp.tile([128, 512], F32, tag="fo")
                nc.vector.tensor_copy(fo[:], stt["tile"][:])
                nc.sync.dma_start(out_d[c2 * 128 : (c2 + 1) * 128, :], fo[:])
                stt["c2"] = None

        pending_norm = []
        for p in range(2):
            for ic in range(4):
                isl = slice(ic * 512, (ic + 1) * 512)
                po = [
                    ps_o.tile([65, 512], F32, tag="o", name=f"po{hi}")
                    for hi in range(2)
                ]
                h1_backlog = []  # head-1 AVs deferred until its slot frees
                for j in range(16):
                    jsl = slice(j * 128, (j + 1) * 128)
                    if j == 0 and pending_norm:
                        pending_norm[0]()  # sums copies (DVE only)
                    st = ps_s.tile([128, 1024], F32, tag="s")
                    # two K=64 row-tiles run concurrently in the PE array
                    nc.tensor.matmul(
                        st[:, 0:512],
                        lhsT=sb_kT[0:64, p, jsl],
                        rhs=sb_qT[0:64, p, isl],
                        start=True,
                        stop=True,
                    )
                    nc.tensor.matmul(
                        st[:, 512:1024],
                        lhsT=sb_kT[64:128, p, jsl],
                        rhs=sb_qT[64:128, p, isl],
                        start=True,
                        stop=True,
                    )
                    if pending_norm:
                        if j == 1:
                            pending_norm[1]()  # bcast+recip+mul head 0
                        elif j == 2:
                            pending_norm[2]()  # ... head 1
                            pending_norm = []
                    # extras: deferred matmuls keep PE fed; x-slot is needed
                    # by the norm broadcasts at j=1,2 so extras wait till j>=3
                    if p == 0 and ic == 0:
                        if 0 < j < 4:
                            emit_qk_chunk(sb_wk, sb_kT, 0, j, ("ps_o", "o"))
                        if j == 0:
                            emit_v_chunk(0)
                            emit_v_chunk(1)
                        elif j < 15:
                            emit_v_chunk(j + 1)
                    elif j == 0 or j >= 3:
                        if p == 0:
                            step_pending_qk()
                        else:
                            step_pending_proj()
                    pt = ptp.tile([128, 1024], BF16, tag="pt")
                    nc.scalar.activation(pt[:], st[:], AF.Exp, scale=SCALE)

                    def emit_av(hi, jj, ptt):
                        nc.tensor.matmul(
                            po[hi][:],
                            lhsT=sb_v[:, jj, 2 * p + hi, 0:65],
                            rhs=ptt[:, hi * 512 : (hi + 1) * 512],
                            start=(jj == 0),
                            stop=(jj == 15),
                            skip_group_check=True,
                        )

                    emit_av(0, j, pt)
                    if pending_norm and j < 2:
                        # head-1 slot frees only after the deferred mul0;
                        # queue its AVs until j==2
                        h1_backlog.append((j, pt))
                    else:
                        for jj, ptt in h1_backlog:
                            emit_av(1, jj, ptt)
                        h1_backlog = []
                        emit_av(1, j, pt)
                pending_norm = make_norm_steps(p, ic, po)
                if p == 1 and ic > 0:
                    # this ic's slice of the output projection; overlaps the
                    # next block's attention
                    pending_proj.extend(range(4 * (ic - 1), 4 * ic))

        # ---- tail: last normalize + remaining projection chunks ----
        for step in pending_norm:
            step()
        while pending_proj or proj_state["c2"] is not None:
            step_pending_proj()
        for c2 in range(12, 16):
            emit_proj_chunk(c2, tag="o" if c2 % 2 else "x")


def _build():
    nc = bacc.Bacc("TRN2", target_bir_lowering=False, debug=False, num_devices=N_CORES)
    xT = nc.dram_tensor("xT", [DM, S], BF16, kind="ExternalInput")
    wq = nc.dram_tensor("wq", [DM, DQ], BF16, kind="ExternalInput")
    wk = nc.dram_tensor("wk", [DM, DQ], BF16, kind="ExternalInput")
    wv = nc.dram_tensor("wv", [DM, DQ], BF16, kind="ExternalInput")
    wo = nc.dram_tensor("wo", [DQ, DM], BF16, kind="ExternalInput")
    out = nc.dram_tensor("out", [S, DM], F32, kind="ExternalOutput")
    with tile.TileContext(nc) as tc:
        _kernel_body(tc, xT.ap(), wq.ap(), wk.ap(), wv.ap(), wo.ap(), out.ap())
    nc.compile()
    return nc


def get_nc():
    global _CACHED_NC
    if _CACHED_NC is None:
        _CACHED_NC = _build()
    return _CACHED_NC


def _in_maps(hidden_states, Wq, Wk, Wv, Wo):
    bf = ml_dtypes.bfloat16
    maps = []
    for c in range(N_CORES):
        b, g = c // 2, c % 2
        cols = slice(g * DQ, (g + 1) * DQ)
        maps.append(
            {
                "xT": np.ascontiguousarray(hidden_states[b].T).astype(bf),
                "wq": np.ascontiguousarray(Wq[:, cols]).astype(bf),
                "wk": np.ascontiguousarray(Wk[:, cols]).astype(bf),
                "wv": np.ascontiguousarray(Wv[:, cols]).astype(bf),
                "wo": np.ascontiguousarray(Wo[cols, :]).astype(bf),
            }
        )
    return maps


def _ensure_profile_support():
    """Best-effort: register the axon NTFF profiling hook + defang the
    bucket upload (zero-egress container). Without this, trace=True dies
    on a missing ``antenv.axon_hooks`` module in this image."""
    import types

    try:
        import antenv

        if "antenv.axon_hooks" not in sys.modules:
            mod = types.ModuleType("antenv.axon_hooks")
            _h = {"hook": None}
            mod.set_axon_ntff_profile_hook = lambda h: _h.__setitem__("hook", h)
            mod.get_axon_ntff_profile_hook = lambda: _h["hook"]
            sys.modules["antenv.axon_hooks"] = mod
            antenv.axon_hooks = mod
        import antenv.axon_hooks as ah

        if ah.get_axon_ntff_profile_hook() is None:
            if "/root/.axon_site" not in sys.path:
                sys.path.append("/root/.axon_site")
            from trn_agent_boot.trn_boot import _ntff_profile_via_ctypes

            hook = _ntff_profile_via_ctypes("/opt/axon/libaxon_pjrt.so")
            if hook is not None:
                ah.set_axon_ntff_profile_hook(hook)
    except Exception:
        pass
    try:
        import concourse.bass_utils as bu

        bu.upload_artifacts = lambda tmpdir: tmpdir
    except Exception:
        pass


def kernel(hidden_states, Wq, Wk, Wv, Wo):
    global LAST_EXEC_TIME_NS, LAST_RESULT
    hidden_states = np.asarray(hidden_states, dtype=np.float32)
    Wq, Wk, Wv, Wo = (np.asarray(w, dtype=np.float32) for w in (Wq, Wk, Wv, Wo))

    trace = bool(os.environ.get("BASS_TRACE"))
    if trace:
        _ensure_profile_support()
    nc = get_nc()
    maps = _in_maps(hidden_states, Wq, Wk, Wv, Wo)
    res = run_bass_kernel_spmd(
        nc,
        maps,
        core_ids=list(range(N_CORES)),
        trace=trace,
        tmpdir=os.environ.get("BASS_TRACE_DIR") or None,
    )
    LAST_RESULT = res
    LAST_EXEC_TIME_NS = res.exec_time_ns

    out = np.empty((B, S, DM), dtype=np.float32)
    for b in range(B):
        out[b] = res.results[2 * b]["out"] + res.results[2 * b + 1]["out"]
    return out


if __name__ == "__main__":
    rng = np.random.default_rng(0)
    hs = rng.standard_normal((B, S, DM), dtype=np.float32)
    ws = [
        (rng.standard_normal((DM, DM), dtype=np.float32) / np.sqrt(DM))
        for _ in range(4)
    ]
    o = kernel(hs, *ws)
    print("out", o.shape, o.dtype, float(np.abs(o).mean()))
    print("exec_time_ns", LAST_EXEC_TIME_NS)


# revision 24
# speedup vs baseline: 1.1245x; 1.0052x over previous
"""Multi-head attention (B=4, S=2048, H=8, Dh=64, Dm=512) on 8 TRN2 NeuronCores.

Sharding: batch*head parallel. Core c owns batch b = c//2 and head group
g = c%2 (4 heads each). Each core computes QKV projection for its head
group, transposed-scores flash-style attention (no max subtraction --
scores ~ N(0,1) after 1/sqrt(Dh) scaling, exp is safe in fp32/bf16), and
its partial output projection against its 256 rows of Wo. The host sums
the two partial projections per batch.

Device-side layout notes:
  - X^T (bf16) is prepared on host so every matmul contracts over the
    partition dim directly.
  - Scores are computed transposed (S^T[j,i] = K Q^T) so the attention*V
    matmul needs no transposition; the two heads of a 128-row Q^T/K^T
    chunk are packed into the PE array as two K=64 row-tiles (tile_position
    (0,0)/(64,0)) running concurrently.
  - Row sums of exp(scores) come for free from a ones-column appended to V
    (M=65 stationary); normalization uses a K=1 broadcast matmul + DVE
    reciprocal/multiply.
"""

import os
import sys

for _p in ("/opt/trn_rl_repo",):
    if os.path.isdir(_p) and _p not in sys.path:
        sys.path.append(_p)

import ml_dtypes
import numpy as np

import concourse.bass as bass
import concourse.tile as tile
from concourse import bacc, mybir
from concourse.bass_utils import run_bass_kernel_spmd

BF16 = mybir.dt.bfloat16
F16 = mybir.dt.float16
F32 = mybir.dt.float32

B, S, DM = 4, 2048, 512
H, DH = 8, 64
HPC = 4  # heads per core
DQ = HPC * DH  # 256: per-core slice of the inner dim
N_CORES = 8
SCALE = DH**-0.5

AF = mybir.ActivationFunctionType

# exported for test harnesses
LAST_EXEC_TIME_NS = None
LAST_RESULT = None

_CACHED_NC = None


def _kernel_body(tc, xT_d, wq_d, wk_d, wv_d, wo_d, out_d):
    from contextlib import ExitStack

    nc = tc.nc
    with ExitStack() as ctx:
        consts = ctx.enter_context(tc.tile_pool(name="consts", bufs=1))
        ptp = ctx.enter_context(tc.tile_pool(name="pt", bufs=6))
        normp = ctx.enter_context(tc.tile_pool(name="norm", bufs=2))
        foutp = ctx.enter_context(tc.tile_pool(name="fout", bufs=3))
        # PSUM: "s" 2x[128,1024]=4 banks, "o" 2x[65,512]=2, "b" 1, "x" 1 -> 8
        ps_s = ctx.enter_context(tc.tile_pool(name="ps_s", bufs=2, space="PSUM"))
        ps_o = ctx.enter_context(tc.tile_pool(name="ps_o", bufs=3, space="PSUM"))
        ps_x = ctx.enter_context(tc.tile_pool(name="ps_x", bufs=1, space="PSUM"))
        drp = ctx.enter_context(tc.tile_pool(name="dram", bufs=2, space="DRAM"))

        sb_xT = consts.tile([128, 4, S], BF16)  # X^T: k-chunk c -> [:, c, :]
        sb_wq = consts.tile([128, 4, DQ], BF16)
        sb_wk = consts.tile([128, 4, DQ], BF16)
        sb_wv = consts.tile([128, 4, DQ], BF16)
        sb_wo = consts.tile([128, 2, DM], BF16)  # d'-chunk p -> [:, p, :]
        sb_qT = consts.tile([128, 2, S], BF16)  # dq-chunk (head pair) p
        sb_kT = consts.tile([128, 2, S], BF16)
        sb_v = consts.tile([128, 16, HPC, 66], BF16)  # V_aug; col 64 = ones
        sb_oT = consts.tile([128, 2, S], BF16)  # normalized O^T
        sb_warm = consts.tile([128, 512], BF16)  # PE warmup fodder
        sb_one = consts.tile([128, 64], F16)  # all-ones (bcast stationary)

        nc.vector.memset(sb_one[:], 1.0)
        nc.vector.memset(sb_v[:, :, :, 64:66], 1.0)
        nc.vector.memset(sb_warm[:], 1.0)
        for w_d, w_sb in ((wq_d, sb_wq), (wk_d, sb_wk), (wv_d, sb_wv)):
            nc.sync.dma_start(w_sb[:], w_d.rearrange("(c p) d -> p c d", p=128))
        nc.sync.dma_start(sb_wo[:], wo_d.rearrange("(c p) d -> p c d", p=128))
        xT_r = xT_d.rearrange("(c p) s -> c p s", p=128)
        for kc in range(4):
            nc.sync.dma_start(sb_xT[:, kc, :], xT_r[kc])

        # Warm the PE (HAM un-throttle needs ~3.4us of sustained matmul) and
        # preload the exp table while the xT DMA is in flight. Enough dummy
        # matmuls to keep PE busy until the DMA lands (else the MID window
        # re-throttles it right before the real work starts).
        pw = ps_x.tile([128, 512], F32, tag="x")
        for r in range(24):
            nc.tensor.matmul(
                pw[:], lhsT=sb_warm[:, 0:128], rhs=sb_warm[:], start=True, stop=True
            )
        warm_act = normp.tile([1, 4], F32, tag="wact")
        nc.scalar.activation(warm_act[:], pw[0:1, 0:4], AF.Exp, scale=-1.0)

        def emit_qk_chunk(w_sb, dst_sb, p, c, pool_tag=("ps_s", "s")):
            """One [128,512] chunk of Q^T or K^T for head-pair p."""
            isl = slice(c * 512, (c + 1) * 512)
            pool = {"ps_s": ps_s, "ps_o": ps_o, "ps_x": ps_x}[pool_tag[0]]
            pq = pool.tile([128, 512], F32, tag=pool_tag[1], name="pqk")
            for kc in range(4):
                nc.tensor.matmul(
                    pq[:],
                    lhsT=w_sb[:, kc, p * 128 : (p + 1) * 128],
                    rhs=sb_xT[:, kc, isl],
                    start=(kc == 0),
                    stop=(kc == 3),
                )
            nc.vector.tensor_copy(dst_sb[:, p, isl], pq[:])

        def emit_qk_chunk_mm(w_sb, p, c, kc, pq):
            nc.tensor.matmul(
                pq[:],
                lhsT=w_sb[:, kc, p * 128 : (p + 1) * 128],
                rhs=sb_xT[:, kc, c * 512 : (c + 1) * 512],
                start=(kc == 0),
                stop=(kc == 3),
            )

        def emit_v_chunk(sc):
            """V natural [s,dv] for s-chunk sc (all 4 heads)."""
            pv = ps_x.tile([128, DQ], F32, tag="x", name="pv")
            for kc in range(4):
                nc.tensor.matmul(
                    pv[:],
                    lhsT=sb_xT[:, kc, sc * 128 : (sc + 1) * 128],
                    rhs=sb_wv[:, kc, :],
                    start=(kc == 0),
                    stop=(kc == 3),
                )
            nc.vector.tensor_copy(
                sb_v[:, sc, :, 0:64], pv.rearrange("p (h d) -> p h d", h=HPC)
            )

        def emit_proj_chunk(c2, tag="x"):
            pf = ps_x.tile([128, 512], F32, tag=tag, name="pf") if tag == "x" else (
                ps_o.tile([128, 512], F32, tag=tag, name="pf2")
            )
            for p in range(2):
                nc.tensor.matmul(
                    pf[:],
                    lhsT=sb_oT[:, p, c2 * 128 : (c2 + 1) * 128],
                    rhs=sb_wo[:, p, :],
                    start=(p == 0),
                    stop=(p == 1),
                )
            fo = foutp.tile([128, 512], F32, tag="fo")
            nc.vector.tensor_copy(fo[:], pf[:])
            nc.sync.dma_start(out_d[c2 * 128 : (c2 + 1) * 128, :], fo[:])

        # ---- lead: Q^T (all chunks) for pair 0 + first K^T chunk; the
        # remaining K^T chunks and all V chunks stream inside block 0 ----
        for c in range(4):
            emit_qk_chunk(sb_wq, sb_qT, 0, c, ("ps_s", "s") if c % 2 else ("ps_o", "o"))
        emit_qk_chunk(sb_wk, sb_kT, 0, 0)

        # deferred work interleaved into attention blocks, one MM per j-iter
        pending_qk = []  # (w_sb, dst_sb, p, c) flattened to per-MM granularity
        for c in range(4):
            pending_qk.append((sb_wk, sb_kT, 1, c))
        for c in range(4):
            pending_qk.append((sb_wq, sb_qT, 1, c))
        qk_state = {"chunk": None, "tile": None, "kc": 0}

        def step_pending_qk():
            stt = qk_state
            if stt["chunk"] is None:
                if not pending_qk:
                    return
                stt["chunk"] = pending_qk.pop(0)
                stt["tile"] = ps_x.tile([128, 512], F32, tag="x", name="pqk1")
                stt["kc"] = 0
            w_sb, dst_sb, p, c = stt["chunk"]
            emit_qk_chunk_mm(w_sb, p, c, stt["kc"], stt["tile"])
            stt["kc"] += 1
            if stt["kc"] == 4:
                nc.vector.tensor_copy(
                    dst_sb[:, p, c * 512 : (c + 1) * 512], stt["tile"][:]
                )
                stt["chunk"] = None

        # ---- attention: pair 0 then pair 1 ----
        # Normalization of block k is emitted lazily, interleaved into the
        # first iterations of block k+1, so the in-order PE stream never
        # stalls long enough for HAM to re-throttle the clock.
        def make_norm_steps(p, ic, po):
            """Normalization of a finished block, split into 3 steps that the
            next block interleaves into its first iterations (the fp16 K=1
            broadcast matmuls sit behind fresh scores in PE order, so the PE
            never stalls waiting on the DVE sums copies)."""
            isl = slice(ic * 512, (ic + 1) * 512)
            held = {}

            def step_sums():
                for hi in range(2):
                    s = normp.tile([65, 512], F16, tag="sums", name=f"sums{hi}")
                    nc.vector.tensor_copy(s[64:65, :], po[hi][64:65, :])
                    held[hi] = s

            def step_head(hi):
                pb = ps_x.tile([64, 512], F32, tag="x", name=f"pb{hi}")
                nc.tensor.matmul(
                    pb[:],
                    lhsT=sb_one[64:65, :],
                    rhs=held[hi][64:65, :],
                    start=True,
                    stop=True,
                )
                rec = normp.tile([64, 512], F32, tag="rec", name=f"rec{hi}")
                nc.vector.reciprocal_approx_fast(rec[:], pb[:])
                if hi == 0:
                    nc.vector.tensor_mul(
                        sb_oT[0:64, p, isl], po[0][0:64, :], rec[:]
                    )
                else:
                    tmpb = normp.tile([64, 512], BF16, tag="tmpb")
                    nc.vector.tensor_mul(tmpb[:], po[1][0:64, :], rec[:])
                    nc.sync.dma_start(sb_oT[64:128, p, isl], tmpb[:])

            return [step_sums, lambda: step_head(0), lambda: step_head(1)]

        # per-MM-granularity deferred projection chunks (run during p1 blocks)
        pending_proj = []
        proj_state = {"c2": None, "tile": None, "p": 0}

        def step_pending_proj():
            stt = proj_state
            if stt["c2"] is None:
                if not pending_proj:
                    return
                stt["c2"] = pending_proj.pop(0)
                stt["tile"] = ps_x.tile([128, 512], F32, tag="x", name="pf")
                stt["p"] = 0
            c2, p = stt["c2"], stt["p"]
            nc.tensor.matmul(
                stt["tile"][:],
                lhsT=sb_oT[:, p, c2 * 128 : (c2 + 1) * 128],
                rhs=sb_wo[:, p, :],
                start=(p == 0),
                stop=(p == 1),
            )
            stt["p"] += 1
            if stt["p"] == 2:
                fo = foutp.tile([128, 512], F32, tag="fo")
                nc.vector.tensor_copy(fo[:], stt["tile"][:])
                nc.sync.dma_start(out_d[c2 * 128 : (c2 + 1) * 128, :], fo[:])
                stt["c2"] = None

        pending_norm = []
        for p in range(2):
            for ic in range(4):
                isl = slice(ic * 512, (ic + 1) * 512)
                po = [
                    ps_o.tile([65, 512], F32, tag="o", name=f"po{hi}")
                    for hi in range(2)
                ]
                if p == 1 and ic > 0:
                    # previous ic's projection slice; its oT inputs complete
                    # during this block's first two iterations (lazy norm)
                    pending_proj.extend(range(4 * (ic - 1), 4 * ic))
                h1_backlog = []  # head-1 AVs deferred until its slot frees
                for j in range(16):
                    jsl = slice(j * 128, (j + 1) * 128)
                    if j == 0 and pending_norm:
                        pending_norm[0]()  # sums copies (DVE only)
                    st = ps_s.tile([128, 1024], F32, tag="s")
                    # two K=64 row-tiles run concurrently in the PE array
                    nc.tensor.matmul(
                        st[:, 0:512],
                        lhsT=sb_kT[0:64, p, jsl],
                        rhs=sb_qT[0:64, p, isl],
                        start=True,
                        stop=True,
                    )
                    nc.tensor.matmul(
                        st[:, 512:1024],
                        lhsT=sb_kT[64:128, p, jsl],
                        rhs=sb_qT[64:128, p, isl],
                        start=True,
                        stop=True,
                    )
                    if pending_norm:
                        if j == 1:
                            pending_norm[1]()  # bcast+recip+mul head 0
                        elif j == 2:
                            pending_norm[2]()  # ... head 1
                            pending_norm = []
                    # extras: deferred matmuls keep PE fed; x-slot is needed
                    # by the norm broadcasts at j=1,2 so extras wait till j>=3
                    if p == 0 and ic == 0:
                        if 0 < j < 4:
                            emit_qk_chunk(sb_wk, sb_kT, 0, j, ("ps_o", "o"))
                        if j == 0:
                            emit_v_chunk(0)
                            emit_v_chunk(1)
                        elif j < 15:
                            emit_v_chunk(j + 1)
                    elif j >= 3:
                        if p == 0:
                            step_pending_qk()
                        else:
                            step_pending_proj()
                    pt = ptp.tile([128, 1024], BF16, tag="pt")
                    nc.scalar.activation(pt[:], st[:], AF.Exp, scale=SCALE)

                    def emit_av(hi, jj, ptt):
                        nc.tensor.matmul(
                            po[hi][:],
                            lhsT=sb_v[:, jj, 2 * p + hi, 0:65],
                            rhs=ptt[:, hi * 512 : (hi + 1) * 512],
                            start=(jj == 0),
                            stop=(jj == 15),
                            skip_group_check=True,
                        )

                    emit_av(0, j, pt)
                    if pending_norm and j < 2:
                        # head-1 slot frees only after the deferred mul0;
                        # queue its AVs until j==2
                        h1_backlog.append((j, pt))
                    else:
                        for jj, ptt in h1_backlog:
                            emit_av(1, jj, ptt)
                        h1_backlog = []
                        emit_av(1, j, pt)
                pending_norm = make_norm_steps(p, ic, po)

        # ---- tail: last normalize + remaining projection chunks ----
        for step in pending_norm:
            step()
        while pending_proj or proj_state["c2"] is not None:
            step_pending_proj()
        for c2 in range(12, 16):
            emit_proj_chunk(c2, tag="o" if c2 % 2 else "x")


def _build():
    nc = bacc.Bacc("TRN2", target_bir_lowering=False, debug=False, num_devices=N_CORES)
    xT = nc.dram_tensor("xT", [DM, S], BF16, kind="ExternalInput")
    wq = nc.dram_tensor("wq", [DM, DQ], BF16, kind="ExternalInput")
    wk = nc.dram_tensor("wk", [DM, DQ], BF16, kind="ExternalInput")
    wv = nc.dram_tensor("wv", [DM, DQ], BF16, kind="ExternalInput")
    wo = nc.dram_tensor("wo", [DQ, DM], BF16, kind="ExternalInput")
    out = nc.dram_tensor("out", [S, DM], F32, kind="ExternalOutput")
    with tile.TileContext(nc) as tc:
        _kernel_body(tc, xT.ap(), wq.ap(), wk.ap(), wv.ap(), wo.ap(), out.ap())
    nc.compile()
    return nc


def get_nc():
    global _CACHED_NC
    if _CACHED_NC is None:
        _CACHED_NC = _build()
    return _CACHED_NC


def _in_maps(hidden_states, Wq, Wk, Wv, Wo):
    bf = ml_dtypes.bfloat16
    maps = []
    for c in range(N_CORES):
        b, g = c // 2, c % 2
        cols = slice(g * DQ, (g + 1) * DQ)
        maps.append(
            {
                "xT": np.ascontiguousarray(hidden_states[b].T).astype(bf),
                "wq": np.ascontiguousarray(Wq[:, cols]).astype(bf),
                "wk": np.ascontiguousarray(Wk[:, cols]).astype(bf),
                "wv": np.ascontiguousarray(Wv[:, cols]).astype(bf),
                "wo": np.ascontiguousarray(Wo[cols, :]).astype(bf),
            }
        )
    return maps


def _ensure_profile_support():
    """Best-effort: register the axon NTFF profiling hook + defang the
    bucket upload (zero-egress container). Without this, trace=True dies
    on a missing ``antenv.axon_hooks`` module in this image."""
    import types

    try:
        import antenv

        if "antenv.axon_hooks" not in sys.modules:
            mod = types.ModuleType("antenv.axon_hooks")
            _h = {"hook": None}
            mod.set_axon_ntff_profile_hook = lambda h: _h.__setitem__("hook", h)
            mod.get_axon_ntff_profile_hook = lambda: _h["hook"]
            sys.modules["antenv.axon_hooks"] = mod
            antenv.axon_hooks = mod
        import antenv.axon_hooks as ah

        if ah.get_axon_ntff_profile_hook() is None:
            if "/root/.axon_site" not in sys.path:
                sys.path.append("/root/.axon_site")
            from trn_agent_boot.trn_boot import _ntff_profile_via_ctypes

            hook = _ntff_profile_via_ctypes("/opt/axon/libaxon_pjrt.so")
            if hook is not None:
                ah.set_axon_ntff_profile_hook(hook)
    except Exception:
        pass
    try:
        import concourse.bass_utils as bu

        bu.upload_artifacts = lambda tmpdir: tmpdir
    except Exception:
        pass


def kernel(hidden_states, Wq, Wk, Wv, Wo):
    global LAST_EXEC_TIME_NS, LAST_RESULT
    hidden_states = np.asarray(hidden_states, dtype=np.float32)
    Wq, Wk, Wv, Wo = (np.asarray(w, dtype=np.float32) for w in (Wq, Wk, Wv, Wo))

    trace = bool(os.environ.get("BASS_TRACE"))
    if trace:
        _ensure_profile_support()
    nc = get_nc()
    maps = _in_maps(hidden_states, Wq, Wk, Wv, Wo)
    res = run_bass_kernel_spmd(
        nc,
        maps,
        core_ids=list(range(N_CORES)),
        trace=trace,
        tmpdir=os.environ.get("BASS_TRACE_DIR") or None,
    )
    LAST_RESULT = res
    LAST_EXEC_TIME_NS = res.exec_time_ns

    out = np.empty((B, S, DM), dtype=np.float32)
    for b in range(B):
        out[b] = res.results[2 * b]["out"] + res.results[2 * b + 1]["out"]
    return out


if __name__ == "__main__":
    rng = np.random.default_rng(0)
    hs = rng.standard_normal((B, S, DM), dtype=np.float32)
    ws = [
        (rng.standard_normal((DM, DM), dtype=np.float32) / np.sqrt(DM))
        for _ in range(4)
    ]
    o = kernel(hs, *ws)
    print("out", o.shape, o.dtype, float(np.abs(o).mean()))
    print("exec_time_ns", LAST_EXEC_TIME_NS)


# revision 26
# speedup vs baseline: 1.1251x; 1.0006x over previous
"""Multi-head attention (B=4, S=2048, H=8, Dh=64, Dm=512) on 8 TRN2 NeuronCores.

Sharding: batch*head parallel. Core c owns batch b = c//2 and head group
g = c%2 (4 heads each). Each core computes QKV projection for its head
group, transposed-scores flash-style attention (no max subtraction --
scores ~ N(0,1) after 1/sqrt(Dh) scaling, exp is safe in fp32/bf16), and
its partial output projection against its 256 rows of Wo. The host sums
the two partial projections per batch.

Device-side layout notes:
  - X^T (bf16) is prepared on host so every matmul contracts over the
    partition dim directly.
  - Scores are computed transposed (S^T[j,i] = K Q^T) so the attention*V
    matmul needs no transposition; the two heads of a 128-row Q^T/K^T
    chunk are packed into the PE array as two K=64 row-tiles (tile_position
    (0,0)/(64,0)) running concurrently.
  - Row sums of exp(scores) come for free from a ones-column appended to V
    (M=65 stationary); normalization uses a K=1 broadcast matmul + DVE
    reciprocal/multiply.
"""

import os
import sys

for _p in ("/opt/trn_rl_repo",):
    if os.path.isdir(_p) and _p not in sys.path:
        sys.path.append(_p)

import ml_dtypes
import numpy as np

import concourse.bass as bass
import concourse.tile as tile
from concourse import bacc, mybir
from concourse.bass_utils import run_bass_kernel_spmd

BF16 = mybir.dt.bfloat16
F16 = mybir.dt.float16
F32 = mybir.dt.float32

B, S, DM = 4, 2048, 512
H, DH = 8, 64
HPC = 4  # heads per core
DQ = HPC * DH  # 256: per-core slice of the inner dim
N_CORES = 8
SCALE = DH**-0.5

AF = mybir.ActivationFunctionType

# exported for test harnesses
LAST_EXEC_TIME_NS = None
LAST_RESULT = None

_CACHED_NC = None


def _kernel_body(tc, xT_d, wq_d, wk_d, wv_d, wo_d, out_d):
    from contextlib import ExitStack

    nc = tc.nc
    with ExitStack() as ctx:
        consts = ctx.enter_context(tc.tile_pool(name="consts", bufs=1))
        ptp = ctx.enter_context(tc.tile_pool(name="pt", bufs=6))
        normp = ctx.enter_context(tc.tile_pool(name="norm", bufs=2))
        foutp = ctx.enter_context(tc.tile_pool(name="fout", bufs=3))
        # PSUM: "s" 2x[128,1024]=4 banks, "o" 2x[65,512]=2, "b" 1, "x" 1 -> 8
        ps_s = ctx.enter_context(tc.tile_pool(name="ps_s", bufs=2, space="PSUM"))
        ps_o = ctx.enter_context(tc.tile_pool(name="ps_o", bufs=3, space="PSUM"))
        ps_x = ctx.enter_context(tc.tile_pool(name="ps_x", bufs=1, space="PSUM"))
        drp = ctx.enter_context(tc.tile_pool(name="dram", bufs=2, space="DRAM"))

        sb_xT = consts.tile([128, 4, S], BF16)  # X^T: k-chunk c -> [:, c, :]
        sb_wq = consts.tile([128, 4, DQ], BF16)
        sb_wk = consts.tile([128, 4, DQ], BF16)
        sb_wv = consts.tile([128, 4, DQ], BF16)
        sb_wo = consts.tile([128, 2, DM], BF16)  # d'-chunk p -> [:, p, :]
        sb_qT = consts.tile([128, 2, S], BF16)  # dq-chunk (head pair) p
        sb_kT = consts.tile([128, 2, S], BF16)
        sb_v = consts.tile([128, 16, HPC, 66], BF16)  # V_aug; col 64 = ones
        sb_oT = consts.tile([128, 2, S], BF16)  # normalized O^T
        sb_warm = consts.tile([128, 512], BF16)  # PE warmup fodder
        sb_one = consts.tile([128, 64], F16)  # all-ones (bcast stationary)

        nc.vector.memset(sb_one[:], 1.0)
        nc.vector.memset(sb_v[:, :, :, 64:66], 1.0)
        nc.vector.memset(sb_warm[:], 1.0)
        for w_d, w_sb in ((wq_d, sb_wq), (wk_d, sb_wk), (wv_d, sb_wv)):
            nc.sync.dma_start(w_sb[:], w_d.rearrange("(c p) d -> p c d", p=128))
        nc.sync.dma_start(sb_wo[:], wo_d.rearrange("(c p) d -> p c d", p=128))
        xT_r = xT_d.rearrange("(c p) s -> c p s", p=128)
        for kc in range(4):
            nc.sync.dma_start(sb_xT[:, kc, :], xT_r[kc])

        # Warm the PE (HAM un-throttle needs ~3.4us of sustained matmul) and
        # preload the exp table while the xT DMA is in flight. Enough dummy
        # matmuls to keep PE busy until the DMA lands (else the MID window
        # re-throttles it right before the real work starts).
        pw = ps_x.tile([128, 512], F32, tag="x")
        for r in range(18):
            nc.tensor.matmul(
                pw[:], lhsT=sb_warm[:, 0:128], rhs=sb_warm[:], start=True, stop=True
            )
        warm_act = normp.tile([1, 4], F32, tag="wact")
        nc.scalar.activation(warm_act[:], pw[0:1, 0:4], AF.Exp, scale=-1.0)

        def emit_qk_chunk(w_sb, dst_sb, p, c, pool_tag=("ps_s", "s")):
            """One [128,512] chunk of Q^T or K^T for head-pair p."""
            isl = slice(c * 512, (c + 1) * 512)
            pool = {"ps_s": ps_s, "ps_o": ps_o, "ps_x": ps_x}[pool_tag[0]]
            pq = pool.tile([128, 512], F32, tag=pool_tag[1], name="pqk")
            for kc in range(4):
                nc.tensor.matmul(
                    pq[:],
                    lhsT=w_sb[:, kc, p * 128 : (p + 1) * 128],
                    rhs=sb_xT[:, kc, isl],
                    start=(kc == 0),
                    stop=(kc == 3),
                )
            nc.vector.tensor_copy(dst_sb[:, p, isl], pq[:])

        def emit_qk_chunk_mm(w_sb, p, c, kc, pq):
            nc.tensor.matmul(
                pq[:],
                lhsT=w_sb[:, kc, p * 128 : (p + 1) * 128],
                rhs=sb_xT[:, kc, c * 512 : (c + 1) * 512],
                start=(kc == 0),
                stop=(kc == 3),
            )

        def emit_v_chunk(sc):
            """V natural [s,dv] for s-chunk sc (all 4 heads)."""
            pv = ps_x.tile([128, DQ], F32, tag="x", name="pv")
            for kc in range(4):
                nc.tensor.matmul(
                    pv[:],
                    lhsT=sb_xT[:, kc, sc * 128 : (sc + 1) * 128],
                    rhs=sb_wv[:, kc, :],
                    start=(kc == 0),
                    stop=(kc == 3),
                )
            nc.vector.tensor_copy(
                sb_v[:, sc, :, 0:64], pv.rearrange("p (h d) -> p h d", h=HPC)
            )

        def emit_proj_chunk(c2, tag="x"):
            pf = ps_x.tile([128, 512], F32, tag=tag, name="pf") if tag == "x" else (
                ps_o.tile([128, 512], F32, tag=tag, name="pf2")
            )
            for p in range(2):
                nc.tensor.matmul(
                    pf[:],
                    lhsT=sb_oT[:, p, c2 * 128 : (c2 + 1) * 128],
                    rhs=sb_wo[:, p, :],
                    start=(p == 0),
                    stop=(p == 1),
                )
            fo = foutp.tile([128, 512], F32, tag="fo")
            nc.vector.tensor_copy(fo[:], pf[:])
            nc.sync.dma_start(out_d[c2 * 128 : (c2 + 1) * 128, :], fo[:])

        # ---- lead: Q^T (all chunks) for pair 0 + first K^T chunk; the
        # remaining K^T chunks and all V chunks stream inside block 0 ----
        for c in range(4):
            emit_qk_chunk(sb_wq, sb_qT, 0, c, ("ps_s", "s") if c % 2 else ("ps_o", "o"))
        emit_qk_chunk(sb_wk, sb_kT, 0, 0)

        # deferred work interleaved into attention blocks, one MM per j-iter
        pending_qk = []  # (w_sb, dst_sb, p, c) flattened to per-MM granularity
        for c in range(4):
            pending_qk.append((sb_wk, sb_kT, 1, c))
        for c in range(4):
            pending_qk.append((sb_wq, sb_qT, 1, c))
        qk_state = {"chunk": None, "tile": None, "kc": 0}

        def step_pending_qk():
            stt = qk_state
            if stt["chunk"] is None:
                if not pending_qk:
                    return
                stt["chunk"] = pending_qk.pop(0)
                stt["tile"] = ps_x.tile([128, 512], F32, tag="x", name="pqk1")
                stt["kc"] = 0
            w_sb, dst_sb, p, c = stt["chunk"]
            emit_qk_chunk_mm(w_sb, p, c, stt["kc"], stt["tile"])
            stt["kc"] += 1
            if stt["kc"] == 4:
                nc.vector.tensor_copy(
                    dst_sb[:, p, c * 512 : (c + 1) * 512], stt["tile"][:]
                )
                stt["chunk"] = None

        # ---- attention: pair 0 then pair 1 ----
        # Normalization of block k is emitted lazily, interleaved into the
        # first iterations of block k+1, so the in-order PE stream never
        # stalls long enough for HAM to re-throttle the clock.
        def make_norm_steps(p, ic, po):
            """Normalization of a finished block, split into 3 steps that the
            next block interleaves into its first iterations (the fp16 K=1
            broadcast matmuls sit behind fresh scores in PE order, so the PE
            never stalls waiting on the DVE sums copies)."""
            isl = slice(ic * 512, (ic + 1) * 512)
            held = {}

            def step_sums():
                for hi in range(2):
                    s = normp.tile([65, 512], F16, tag="sums", name=f"sums{hi}")
                    nc.vector.tensor_copy(s[64:65, :], po[hi][64:65, :])
                    held[hi] = s

            def step_head(hi):
                pb = ps_x.tile([64, 512], F32, tag="x", name=f"pb{hi}")
                nc.tensor.matmul(
                    pb[:],
                    lhsT=sb_one[64:65, :],
                    rhs=held[hi][64:65, :],
                    start=True,
                    stop=True,
                )
                rec = normp.tile([64, 512], F32, tag="rec", name=f"rec{hi}")
                nc.vector.reciprocal_approx_fast(rec[:], pb[:])
                if hi == 0:
                    nc.vector.tensor_mul(
                        sb_oT[0:64, p, isl], po[0][0:64, :], rec[:]
                    )
                else:
                    tmpb = normp.tile([64, 512], BF16, tag="tmpb")
                    nc.vector.tensor_mul(tmpb[:], po[1][0:64, :], rec[:])
                    nc.sync.dma_start(sb_oT[64:128, p, isl], tmpb[:])

            return [step_sums, lambda: step_head(0), lambda: step_head(1)]

        # per-MM-granularity deferred projection chunks (run during p1 blocks)
        pending_proj = []
        proj_state = {"c2": None, "tile": None, "p": 0}

        def step_pending_proj():
            stt = proj_state
            if stt["c2"] is None:
                if not pending_proj:
                    return
                stt["c2"] = pending_proj.pop(0)
                stt["tile"] = ps_x.tile([128, 512], F32, tag="x", name="pf")
                stt["p"] = 0
            c2, p = stt["c2"], stt["p"]
            nc.tensor.matmul(
                stt["tile"][:],
                lhsT=sb_oT[:, p, c2 * 128 : (c2 + 1) * 128],
                rhs=sb_wo[:, p, :],
                start=(p == 0),
                stop=(p == 1),
            )
            stt["p"] += 1
            if stt["p"] == 2:
                fo = foutp.tile([128, 512], F32, tag="fo")
                nc.vector.tensor_copy(fo[:], stt["tile"][:])
                nc.sync.dma_start(out_d[c2 * 128 : (c2 + 1) * 128, :], fo[:])
                stt["c2"] = None

        pending_norm = []
        blocks = [(p, ic) for p in range(2) for ic in range(4)]

        def emit_scores(p, ic, j):
            isl = slice(ic * 512, (ic + 1) * 512)
            jsl = slice(j * 128, (j + 1) * 128)
            st = ps_s.tile([128, 1024], F32, tag="s")
            nc.tensor.matmul(
                st[:, 0:512],
                lhsT=sb_kT[0:64, p, jsl],
                rhs=sb_qT[0:64, p, isl],
                start=True,
                stop=True,
            )
            nc.tensor.matmul(
                st[:, 512:1024],
                lhsT=sb_kT[64:128, p, jsl],
                rhs=sb_qT[64:128, p, isl],
                start=True,
                stop=True,
            )
            return st

        def emit_exp(st):
            pt = ptp.tile([128, 1024], BF16, tag="pt")
            nc.scalar.activation(pt[:], st[:], AF.Exp, scale=SCALE)
            return pt

        carry_pt = None
        for bi, (p, ic) in enumerate(blocks):
            isl = slice(ic * 512, (ic + 1) * 512)
            po = [
                ps_o.tile([65, 512], F32, tag="o", name=f"po{hi}")
                for hi in range(2)
            ]
            if p == 1 and ic > 0:
                # previous ic's projection slice; its oT inputs complete
                # during this block's first two iterations (lazy norm)
                pending_proj.extend(range(4 * (ic - 1), 4 * ic))
            h1_backlog = []  # head-1 AVs deferred until its slot frees
            for j in range(16):
                if j == 0 and pending_norm:
                    pending_norm[0]()  # sums copies (DVE only)
                used_carry = j == 0 and carry_pt is not None
                if used_carry:
                    pt = carry_pt  # scores+exp already ran in previous block
                    carry_pt = None
                else:
                    st = emit_scores(p, ic, j)
                if pending_norm:
                    if j == 1:
                        pending_norm[1]()  # bcast+recip+mul head 0
                    elif j == 2:
                        pending_norm[2]()  # ... head 1
                        pending_norm = []
                # extras: deferred matmuls keep PE fed; x-slot is needed
                # by the norm broadcasts at j=1,2 so extras wait till j>=3
                if p == 0 and ic == 0:
                    if 0 < j < 4:
                        emit_qk_chunk(sb_wk, sb_kT, 0, j, ("ps_o", "o"))
                    if j == 0:
                        emit_v_chunk(0)
                        emit_v_chunk(1)
                    elif j < 15:
                        emit_v_chunk(j + 1)
                elif j >= 3:
                    if p == 0:
                        step_pending_qk()
                    else:
                        step_pending_proj()
                if not used_carry:
                    pt = emit_exp(st)

                def emit_av(hi, jj, ptt):
                    nc.tensor.matmul(
                        po[hi][:],
                        lhsT=sb_v[:, jj, 2 * p + hi, 0:65],
                        rhs=ptt[:, hi * 512 : (hi + 1) * 512],
                        start=(jj == 0),
                        stop=(jj == 15),
                        skip_group_check=True,
                    )

                emit_av(0, j, pt)
                if pending_norm and j < 2:
                    # head-1 slot frees only after the deferred mul0;
                    # queue its AVs until j==2
                    h1_backlog.append((j, pt))
                else:
                    for jj, ptt in h1_backlog:
                        emit_av(1, jj, ptt)
                    h1_backlog = []
                    emit_av(1, j, pt)
                if j == 15 and bi + 1 < len(blocks):
                    # cross-block pipeline: next block's first scores+exp
                    # issue here so ACT rolls straight over the boundary
                    np_, nic = blocks[bi + 1]
                    carry_pt = emit_exp(emit_scores(np_, nic, 0))
            pending_norm = make_norm_steps(p, ic, po)

        # ---- tail: last normalize + remaining projection chunks ----
        for step in pending_norm:
            step()
        while pending_proj or proj_state["c2"] is not None:
            step_pending_proj()
        for c2 in range(12, 16):
            emit_proj_chunk(c2, tag="o" if c2 % 2 else "x")


def _build():
    nc = bacc.Bacc("TRN2", target_bir_lowering=False, debug=False, num_devices=N_CORES)
    xT = nc.dram_tensor("xT", [DM, S], BF16, kind="ExternalInput")
    wq = nc.dram_tensor("wq", [DM, DQ], BF16, kind="ExternalInput")
    wk = nc.dram_tensor("wk", [DM, DQ], BF16, kind="ExternalInput")
    wv = nc.dram_tensor("wv", [DM, DQ], BF16, kind="ExternalInput")
    wo = nc.dram_tensor("wo", [DQ, DM], BF16, kind="ExternalInput")
    out = nc.dram_tensor("out", [S, DM], F32, kind="ExternalOutput")
    with tile.TileContext(nc) as tc:
        _kernel_body(tc, xT.ap(), wq.ap(), wk.ap(), wv.ap(), wo.ap(), out.ap())
    nc.compile()
    return nc


def get_nc():
    global _CACHED_NC
    if _CACHED_NC is None:
        _CACHED_NC = _build()
    return _CACHED_NC


def _in_maps(hidden_states, Wq, Wk, Wv, Wo):
    bf = ml_dtypes.bfloat16
    maps = []
    for c in range(N_CORES):
        b, g = c // 2, c % 2
        cols = slice(g * DQ, (g + 1) * DQ)
        maps.append(
            {
                "xT": np.ascontiguousarray(hidden_states[b].T).astype(bf),
                "wq": np.ascontiguousarray(Wq[:, cols]).astype(bf),
                "wk": np.ascontiguousarray(Wk[:, cols]).astype(bf),
                "wv": np.ascontiguousarray(Wv[:, cols]).astype(bf),
                "wo": np.ascontiguousarray(Wo[cols, :]).astype(bf),
            }
        )
    return maps


def _ensure_profile_support():
    """Best-effort: register the axon NTFF profiling hook + defang the
    bucket upload (zero-egress container). Without this, trace=True dies
    on a missing ``antenv.axon_hooks`` module in this image."""
    import types

    try:
        import antenv

        if "antenv.axon_hooks" not in sys.modules:
            mod = types.ModuleType("antenv.axon_hooks")
            _h = {"hook": None}
            mod.set_axon_ntff_profile_hook = lambda h: _h.__setitem__("hook", h)
            mod.get_axon_ntff_profile_hook = lambda: _h["hook"]
            sys.modules["antenv.axon_hooks"] = mod
            antenv.axon_hooks = mod
        import antenv.axon_hooks as ah

        if ah.get_axon_ntff_profile_hook() is None:
            if "/root/.axon_site" not in sys.path:
                sys.path.append("/root/.axon_site")
            from trn_agent_boot.trn_boot import _ntff_profile_via_ctypes

            hook = _ntff_profile_via_ctypes("/opt/axon/libaxon_pjrt.so")
            if hook is not None:
                ah.set_axon_ntff_profile_hook(hook)
    except Exception:
        pass
    try:
        import concourse.bass_utils as bu

        bu.upload_artifacts = lambda tmpdir: tmpdir
    except Exception:
        pass


def kernel(hidden_states, Wq, Wk, Wv, Wo):
    global LAST_EXEC_TIME_NS, LAST_RESULT
    hidden_states = np.asarray(hidden_states, dtype=np.float32)
    Wq, Wk, Wv, Wo = (np.asarray(w, dtype=np.float32) for w in (Wq, Wk, Wv, Wo))

    trace = bool(os.environ.get("BASS_TRACE"))
    if trace:
        _ensure_profile_support()
    nc = get_nc()
    maps = _in_maps(hidden_states, Wq, Wk, Wv, Wo)
    res = run_bass_kernel_spmd(
        nc,
        maps,
        core_ids=list(range(N_CORES)),
        trace=trace,
        tmpdir=os.environ.get("BASS_TRACE_DIR") or None,
    )
    LAST_RESULT = res
    LAST_EXEC_TIME_NS = res.exec_time_ns

    out = np.empty((B, S, DM), dtype=np.float32)
    for b in range(B):
        out[b] = res.results[2 * b]["out"] + res.results[2 * b + 1]["out"]
    return out


if __name__ == "__main__":
    rng = np.random.default_rng(0)
    hs = rng.standard_normal((B, S, DM), dtype=np.float32)
    ws = [
        (rng.standard_normal((DM, DM), dtype=np.float32) / np.sqrt(DM))
        for _ in range(4)
    ]
    o = kernel(hs, *ws)
    print("out", o.shape, o.dtype, float(np.abs(o).mean()))
    print("exec_time_ns", LAST_EXEC_TIME_NS)


# revision 27
# speedup vs baseline: 1.1752x; 1.0445x over previous
"""Multi-head attention (B=4, S=2048, H=8, Dh=64, Dm=512) on 8 TRN2 NeuronCores.

Sharding: batch*head parallel. Core c owns batch b = c//2 and head group
g = c%2 (4 heads each). Each core computes QKV projection for its head
group, transposed-scores flash-style attention (no max subtraction --
scores ~ N(0,1) after 1/sqrt(Dh) scaling, exp is safe in fp32/bf16), and
its partial output projection against its 256 rows of Wo. The host sums
the two partial projections per batch.

Device-side layout notes:
  - X^T (bf16) is prepared on host so every matmul contracts over the
    partition dim directly.
  - Scores are computed transposed (S^T[j,i] = K Q^T) so the attention*V
    matmul needs no transposition; the two heads of a 128-row Q^T/K^T
    chunk are packed into the PE array as two K=64 row-tiles (tile_position
    (0,0)/(64,0)) running concurrently.
  - Row sums of exp(scores) come for free from a ones-column appended to V
    (M=65 stationary); normalization uses a K=1 broadcast matmul + DVE
    reciprocal/multiply.
"""

import os
import sys

for _p in ("/opt/trn_rl_repo",):
    if os.path.isdir(_p) and _p not in sys.path:
        sys.path.append(_p)

import ml_dtypes
import numpy as np

import concourse.bass as bass
import concourse.tile as tile
from concourse import bacc, mybir
from concourse.bass_utils import run_bass_kernel_spmd

BF16 = mybir.dt.bfloat16
F16 = mybir.dt.float16
F32 = mybir.dt.float32

B, S, DM = 4, 2048, 512
H, DH = 8, 64
HPC = 4  # heads per core
DQ = HPC * DH  # 256: per-core slice of the inner dim
N_CORES = 8
SCALE = DH**-0.5

AF = mybir.ActivationFunctionType

# exported for test harnesses
LAST_EXEC_TIME_NS = None
LAST_RESULT = None

_CACHED_NC = None


def _kernel_body(tc, xT_d, wq_d, wk_d, wv_d, wo_d, out_d):
    from contextlib import ExitStack

    nc = tc.nc
    with ExitStack() as ctx:
        consts = ctx.enter_context(tc.tile_pool(name="consts", bufs=1))
        ptp = ctx.enter_context(tc.tile_pool(name="pt", bufs=6))
        normp = ctx.enter_context(tc.tile_pool(name="norm", bufs=2))
        foutp = ctx.enter_context(tc.tile_pool(name="fout", bufs=3))
        # PSUM: "s" 2x[128,1024]=4 banks, "o" 2x[65,512]=2, "b" 1, "x" 1 -> 8
        ps_s = ctx.enter_context(tc.tile_pool(name="ps_s", bufs=2, space="PSUM"))
        ps_o = ctx.enter_context(tc.tile_pool(name="ps_o", bufs=3, space="PSUM"))
        ps_x = ctx.enter_context(tc.tile_pool(name="ps_x", bufs=1, space="PSUM"))
        drp = ctx.enter_context(tc.tile_pool(name="dram", bufs=2, space="DRAM"))

        sb_xT = consts.tile([128, 4, S], BF16)  # X^T: k-chunk c -> [:, c, :]
        sb_wq = consts.tile([128, 4, DQ], BF16)
        sb_wk = consts.tile([128, 4, DQ], BF16)
        sb_wv = consts.tile([128, 4, DQ], BF16)
        sb_wo = consts.tile([128, 2, DM], BF16)  # d'-chunk p -> [:, p, :]
        sb_qT = consts.tile([128, 2, S], BF16)  # dq-chunk (head pair) p
        sb_kT = consts.tile([128, 2, S], BF16)
        sb_v = consts.tile([128, 16, HPC, 66], BF16)  # V_aug; col 64 = ones
        sb_oT = consts.tile([128, 2, S], BF16)  # normalized O^T
        sb_warm = consts.tile([128, 512], BF16)  # PE warmup fodder
        sb_one = consts.tile([128, 64], F16)  # all-ones (bcast stationary)

        nc.vector.memset(sb_one[:], 1.0)
        nc.vector.memset(sb_v[:, :, :, 64:66], 1.0)
        nc.vector.memset(sb_warm[:], 1.0)
        for w_d, w_sb in ((wq_d, sb_wq), (wk_d, sb_wk), (wv_d, sb_wv)):
            nc.sync.dma_start(w_sb[:], w_d.rearrange("(c p) d -> p c d", p=128))
        nc.sync.dma_start(sb_wo[:], wo_d.rearrange("(c p) d -> p c d", p=128))
        xT_r = xT_d.rearrange("(c p) s -> c p s", p=128)
        for kc in range(4):
            nc.sync.dma_start(sb_xT[:, kc, :], xT_r[kc])

        # Warm the PE (HAM un-throttle needs ~3.4us of sustained matmul) and
        # preload the exp table while the xT DMA is in flight. Enough dummy
        # matmuls to keep PE busy until the DMA lands (else the MID window
        # re-throttles it right before the real work starts).
        pw = ps_x.tile([128, 512], F32, tag="x")
        for r in range(18):
            nc.tensor.matmul(
                pw[:], lhsT=sb_warm[:, 0:128], rhs=sb_warm[:], start=True, stop=True
            )
        warm_act = normp.tile([1, 4], F32, tag="wact")
        nc.scalar.activation(warm_act[:], pw[0:1, 0:4], AF.Exp, scale=-1.0)

        def emit_qk_chunk(w_sb, dst_sb, p, c, pool_tag=("ps_s", "s")):
            """One [128,512] chunk of Q^T or K^T for head-pair p."""
            isl = slice(c * 512, (c + 1) * 512)
            pool = {"ps_s": ps_s, "ps_o": ps_o, "ps_x": ps_x}[pool_tag[0]]
            pq = pool.tile([128, 512], F32, tag=pool_tag[1], name="pqk")
            for kc in range(4):
                nc.tensor.matmul(
                    pq[:],
                    lhsT=w_sb[:, kc, p * 128 : (p + 1) * 128],
                    rhs=sb_xT[:, kc, isl],
                    start=(kc == 0),
                    stop=(kc == 3),
                )
            nc.vector.tensor_copy(dst_sb[:, p, isl], pq[:])

        def emit_qk_chunk_mm(w_sb, p, c, kc, pq):
            nc.tensor.matmul(
                pq[:],
                lhsT=w_sb[:, kc, p * 128 : (p + 1) * 128],
                rhs=sb_xT[:, kc, c * 512 : (c + 1) * 512],
                start=(kc == 0),
                stop=(kc == 3),
            )

        def emit_v_chunk(sc):
            """V natural [s,dv] for s-chunk sc (all 4 heads)."""
            pv = ps_x.tile([128, DQ], F32, tag="x", name="pv")
            for kc in range(4):
                nc.tensor.matmul(
                    pv[:],
                    lhsT=sb_xT[:, kc, sc * 128 : (sc + 1) * 128],
                    rhs=sb_wv[:, kc, :],
                    start=(kc == 0),
                    stop=(kc == 3),
                )
            nc.vector.tensor_copy(
                sb_v[:, sc, :, 0:64], pv.rearrange("p (h d) -> p h d", h=HPC)
            )

        def emit_proj_chunk(c2, tag="x"):
            pf = ps_x.tile([128, 512], F32, tag=tag, name="pf") if tag == "x" else (
                ps_o.tile([128, 512], F32, tag=tag, name="pf2")
            )
            for p in range(2):
                nc.tensor.matmul(
                    pf[:],
                    lhsT=sb_oT[:, p, c2 * 128 : (c2 + 1) * 128],
                    rhs=sb_wo[:, p, :],
                    start=(p == 0),
                    stop=(p == 1),
                )
            fo = foutp.tile([128, 512], F32, tag="fo")
            nc.vector.tensor_copy(fo[:], pf[:])
            nc.sync.dma_start(out_d[c2 * 128 : (c2 + 1) * 128, :], fo[:])

        # ---- lead: Q^T (all chunks) for pair 0 + first K^T chunk; the
        # remaining K^T chunks and all V chunks stream inside block 0 ----
        for c in range(4):
            emit_qk_chunk(sb_wq, sb_qT, 0, c, ("ps_s", "s") if c % 2 else ("ps_o", "o"))
        emit_qk_chunk(sb_wk, sb_kT, 0, 0)

        # deferred work interleaved into attention blocks, one MM per j-iter
        pending_qk = []  # (w_sb, dst_sb, p, c) flattened to per-MM granularity
        for c in range(4):
            pending_qk.append((sb_wk, sb_kT, 1, c))
        for c in range(4):
            pending_qk.append((sb_wq, sb_qT, 1, c))
        qk_state = {"chunk": None, "tile": None, "kc": 0}

        def step_pending_qk():
            stt = qk_state
            if stt["chunk"] is None:
                if not pending_qk:
                    return
                stt["chunk"] = pending_qk.pop(0)
                stt["tile"] = ps_x.tile([128, 512], F32, tag="x", name="pqk1")
                stt["kc"] = 0
            w_sb, dst_sb, p, c = stt["chunk"]
            emit_qk_chunk_mm(w_sb, p, c, stt["kc"], stt["tile"])
            stt["kc"] += 1
            if stt["kc"] == 4:
                nc.vector.tensor_copy(
                    dst_sb[:, p, c * 512 : (c + 1) * 512], stt["tile"][:]
                )
                stt["chunk"] = None

        # ---- attention: pair 0 then pair 1 ----
        # Normalization of block k is emitted lazily, interleaved into the
        # first iterations of block k+1, so the in-order PE stream never
        # stalls long enough for HAM to re-throttle the clock.
        def make_norm_steps(p, ic, po):
            """Normalization of a finished block, split into 3 steps that the
            next block interleaves into its first iterations (the fp16 K=1
            broadcast matmuls sit behind fresh scores in PE order, so the PE
            never stalls waiting on the DVE sums copies)."""
            isl = slice(ic * 512, (ic + 1) * 512)
            held = {}

            def step_sums():
                for hi in range(2):
                    s = normp.tile([65, 512], F16, tag="sums", name=f"sums{hi}")
                    nc.vector.tensor_copy(s[64:65, :], po[hi][64:65, :])
                    held[hi] = s

            def step_head(hi):
                pb = ps_x.tile([64, 512], F32, tag="x", name=f"pb{hi}")
                nc.tensor.matmul(
                    pb[:],
                    lhsT=sb_one[64:65, :],
                    rhs=held[hi][64:65, :],
                    start=True,
                    stop=True,
                )
                rec = normp.tile([64, 512], F32, tag="rec", name=f"rec{hi}")
                nc.vector.reciprocal_approx_fast(rec[:], pb[:])
                if hi == 0:
                    nc.vector.tensor_mul(
                        sb_oT[0:64, p, isl], po[0][0:64, :], rec[:]
                    )
                else:
                    tmpb = normp.tile([64, 512], BF16, tag="tmpb")
                    nc.vector.tensor_mul(tmpb[:], po[1][0:64, :], rec[:])
                    nc.sync.dma_start(sb_oT[64:128, p, isl], tmpb[:])

            return [step_sums, lambda: step_head(0), lambda: step_head(1)]

        # per-MM-granularity deferred projection chunks (run during p1 blocks)
        pending_proj = []
        proj_state = {"c2": None, "tile": None, "p": 0}

        def step_pending_proj():
            stt = proj_state
            if stt["c2"] is None:
                if not pending_proj:
                    return
                stt["c2"] = pending_proj.pop(0)
                stt["tile"] = ps_x.tile([128, 512], F32, tag="x", name="pf")
                stt["p"] = 0
            c2, p = stt["c2"], stt["p"]
            nc.tensor.matmul(
                stt["tile"][:],
                lhsT=sb_oT[:, p, c2 * 128 : (c2 + 1) * 128],
                rhs=sb_wo[:, p, :],
                start=(p == 0),
                stop=(p == 1),
            )
            stt["p"] += 1
            if stt["p"] == 2:
                fo = foutp.tile([128, 512], F32, tag="fo")
                nc.vector.tensor_copy(fo[:], stt["tile"][:])
                nc.sync.dma_start(out_d[c2 * 128 : (c2 + 1) * 128, :], fo[:])
                stt["c2"] = None

        pending_norm = []
        blocks = [(p, ic) for p in range(2) for ic in range(4)]

        def emit_scores(p, ic, j):
            isl = slice(ic * 512, (ic + 1) * 512)
            jsl = slice(j * 128, (j + 1) * 128)
            st = ps_s.tile([128, 1024], F32, tag="s")
            nc.tensor.matmul(
                st[:, 0:512],
                lhsT=sb_kT[0:64, p, jsl],
                rhs=sb_qT[0:64, p, isl],
                start=True,
                stop=True,
            )
            nc.tensor.matmul(
                st[:, 512:1024],
                lhsT=sb_kT[64:128, p, jsl],
                rhs=sb_qT[64:128, p, isl],
                start=True,
                stop=True,
            )
            return st

        def emit_exp(st):
            pt = ptp.tile([128, 1024], BF16, tag="pt")
            nc.scalar.activation(pt[:], st[:], AF.Exp, scale=SCALE)
            return pt

        carry_pt = None
        for bi, (p, ic) in enumerate(blocks):
            isl = slice(ic * 512, (ic + 1) * 512)
            po = [
                ps_o.tile([65, 512], F32, tag="o", name=f"po{hi}")
                for hi in range(2)
            ]
            if p == 1 and ic > 0:
                # previous ic's projection slice; its oT inputs complete
                # during this block's first two iterations (lazy norm)
                pending_proj.extend(range(4 * (ic - 1), 4 * ic))
            pts = []  # pt tile per j (consumed by lagged AVs)
            for j in range(16):
                if j == 0 and pending_norm:
                    pending_norm[0]()  # sums copies (DVE only)
                used_carry = j == 0 and carry_pt is not None
                if used_carry:
                    pt = carry_pt  # scores+exp already ran in previous block
                    carry_pt = None
                else:
                    st = emit_scores(p, ic, j)
                if pending_norm:
                    if j == 1:
                        pending_norm[1]()  # bcast+recip+mul head 0
                    elif j == 2:
                        pending_norm[2]()  # ... head 1
                        pending_norm = []
                # extras: deferred matmuls keep PE fed; x-slot is needed
                # by the norm broadcasts at j=1,2 so extras wait till j>=3
                if p == 0 and ic == 0:
                    if 0 < j < 4:
                        emit_qk_chunk(sb_wk, sb_kT, 0, j, ("ps_o", "o"))
                    if j == 0:
                        emit_v_chunk(0)
                        emit_v_chunk(1)
                    elif j < 15:
                        emit_v_chunk(j + 1)
                elif j >= 3:
                    if p == 0:
                        step_pending_qk()
                    else:
                        step_pending_proj()
                if not used_carry:
                    pt = emit_exp(st)

                def emit_av(hi, jj, ptt):
                    nc.tensor.matmul(
                        po[hi][:],
                        lhsT=sb_v[:, jj, 2 * p + hi, 0:65],
                        rhs=ptt[:, hi * 512 : (hi + 1) * 512],
                        start=(jj == 0),
                        stop=(jj == 15),
                        skip_group_check=True,
                    )

                # uniform AV lag (h0 by 1 iter, h1 by 2) keeps scores ahead
                # of the AV stream so ACT never waits at block boundaries
                pts.append(pt)
                if j >= 1:
                    emit_av(0, j - 1, pts[j - 1])
                if j >= 2:
                    emit_av(1, j - 2, pts[j - 2])
                if j == 15:
                    emit_av(0, 15, pts[15])
                    emit_av(1, 14, pts[14])
                    emit_av(1, 15, pts[15])
                    if bi + 1 < len(blocks):
                        # cross-block pipeline: next block's first scores+exp
                        np_, nic = blocks[bi + 1]
                        carry_pt = emit_exp(emit_scores(np_, nic, 0))
            pending_norm = make_norm_steps(p, ic, po)

        # ---- tail: last normalize + remaining projection chunks ----
        for step in pending_norm:
            step()
        while pending_proj or proj_state["c2"] is not None:
            step_pending_proj()
        for c2 in range(12, 16):
            emit_proj_chunk(c2, tag="o" if c2 % 2 else "x")


def _build():
    nc = bacc.Bacc("TRN2", target_bir_lowering=False, debug=False, num_devices=N_CORES)
    xT = nc.dram_tensor("xT", [DM, S], BF16, kind="ExternalInput")
    wq = nc.dram_tensor("wq", [DM, DQ], BF16, kind="ExternalInput")
    wk = nc.dram_tensor("wk", [DM, DQ], BF16, kind="ExternalInput")
    wv = nc.dram_tensor("wv", [DM, DQ], BF16, kind="ExternalInput")
    wo = nc.dram_tensor("wo", [DQ, DM], BF16, kind="ExternalInput")
    out = nc.dram_tensor("out", [S, DM], F32, kind="ExternalOutput")
    with tile.TileContext(nc) as tc:
        _kernel_body(tc, xT.ap(), wq.ap(), wk.ap(), wv.ap(), wo.ap(), out.ap())
    nc.compile()
    return nc


def get_nc():
    global _CACHED_NC
    if _CACHED_NC is None:
        _CACHED_NC = _build()
    return _CACHED_NC


def _in_maps(hidden_states, Wq, Wk, Wv, Wo):
    bf = ml_dtypes.bfloat16
    maps = []
    for c in range(N_CORES):
        b, g = c // 2, c % 2
        cols = slice(g * DQ, (g + 1) * DQ)
        maps.append(
            {
                "xT": np.ascontiguousarray(hidden_states[b].T).astype(bf),
                "wq": np.ascontiguousarray(Wq[:, cols]).astype(bf),
                "wk": np.ascontiguousarray(Wk[:, cols]).astype(bf),
                "wv": np.ascontiguousarray(Wv[:, cols]).astype(bf),
                "wo": np.ascontiguousarray(Wo[cols, :]).astype(bf),
            }
        )
    return maps


def _ensure_profile_support():
    """Best-effort: register the axon NTFF profiling hook + defang the
    bucket upload (zero-egress container). Without this, trace=True dies
    on a missing ``antenv.axon_hooks`` module in this image."""
    import types

    try:
        import antenv

        if "antenv.axon_hooks" not in sys.modules:
            mod = types.ModuleType("antenv.axon_hooks")
            _h = {"hook": None}
            mod.set_axon_ntff_profile_hook = lambda h: _h.__setitem__("hook", h)
            mod.get_axon_ntff_profile_hook = lambda: _h["hook"]
            sys.modules["antenv.axon_hooks"] = mod
            antenv.axon_hooks = mod
        import antenv.axon_hooks as ah

        if ah.get_axon_ntff_profile_hook() is None:
            if "/root/.axon_site" not in sys.path:
                sys.path.append("/root/.axon_site")
            from trn_agent_boot.trn_boot import _ntff_profile_via_ctypes

            hook = _ntff_profile_via_ctypes("/opt/axon/libaxon_pjrt.so")
            if hook is not None:
                ah.set_axon_ntff_profile_hook(hook)
    except Exception:
        pass
    try:
        import concourse.bass_utils as bu

        bu.upload_artifacts = lambda tmpdir: tmpdir
    except Exception:
        pass


def kernel(hidden_states, Wq, Wk, Wv, Wo):
    global LAST_EXEC_TIME_NS, LAST_RESULT
    hidden_states = np.asarray(hidden_states, dtype=np.float32)
    Wq, Wk, Wv, Wo = (np.asarray(w, dtype=np.float32) for w in (Wq, Wk, Wv, Wo))

    trace = bool(os.environ.get("BASS_TRACE"))
    if trace:
        _ensure_profile_support()
    nc = get_nc()
    maps = _in_maps(hidden_states, Wq, Wk, Wv, Wo)
    res = run_bass_kernel_spmd(
        nc,
        maps,
        core_ids=list(range(N_CORES)),
        trace=trace,
        tmpdir=os.environ.get("BASS_TRACE_DIR") or None,
    )
    LAST_RESULT = res
    LAST_EXEC_TIME_NS = res.exec_time_ns

    out = np.empty((B, S, DM), dtype=np.float32)
    for b in range(B):
        out[b] = res.results[2 * b]["out"] + res.results[2 * b + 1]["out"]
    return out


if __name__ == "__main__":
    rng = np.random.default_rng(0)
    hs = rng.standard_normal((B, S, DM), dtype=np.float32)
    ws = [
        (rng.standard_normal((DM, DM), dtype=np.float32) / np.sqrt(DM))
        for _ in range(4)
    ]
    o = kernel(hs, *ws)
    print("out", o.shape, o.dtype, float(np.abs(o).mean()))
    print("exec_time_ns", LAST_EXEC_TIME_NS)


# revision 28
# speedup vs baseline: 1.2066x; 1.0267x over previous
"""Multi-head attention (B=4, S=2048, H=8, Dh=64, Dm=512) on 8 TRN2 NeuronCores.

Sharding: batch*head parallel. Core c owns batch b = c//2 and head group
g = c%2 (4 heads each). Each core computes QKV projection for its head
group, transposed-scores flash-style attention (no max subtraction --
scores ~ N(0,1) after 1/sqrt(Dh) scaling, exp is safe in fp32/bf16), and
its partial output projection against its 256 rows of Wo. The host sums
the two partial projections per batch.

Device-side layout notes:
  - X^T (bf16) is prepared on host so every matmul contracts over the
    partition dim directly.
  - Scores are computed transposed (S^T[j,i] = K Q^T) so the attention*V
    matmul needs no transposition; the two heads of a 128-row Q^T/K^T
    chunk are packed into the PE array as two K=64 row-tiles (tile_position
    (0,0)/(64,0)) running concurrently.
  - Row sums of exp(scores) come for free from a ones-column appended to V
    (M=65 stationary); normalization uses a K=1 broadcast matmul + DVE
    reciprocal/multiply.
"""

import os
import sys

for _p in ("/opt/trn_rl_repo",):
    if os.path.isdir(_p) and _p not in sys.path:
        sys.path.append(_p)

import ml_dtypes
import numpy as np

import concourse.bass as bass
import concourse.tile as tile
from concourse import bacc, mybir
from concourse.bass_utils import run_bass_kernel_spmd

BF16 = mybir.dt.bfloat16
F16 = mybir.dt.float16
F32 = mybir.dt.float32

B, S, DM = 4, 2048, 512
H, DH = 8, 64
HPC = 4  # heads per core
DQ = HPC * DH  # 256: per-core slice of the inner dim
N_CORES = 8
SCALE = DH**-0.5

AF = mybir.ActivationFunctionType

# exported for test harnesses
LAST_EXEC_TIME_NS = None
LAST_RESULT = None

_CACHED_NC = None


def _kernel_body(tc, xT_d, wq_d, wk_d, wv_d, wo_d, out_d):
    from contextlib import ExitStack

    nc = tc.nc
    with ExitStack() as ctx:
        consts = ctx.enter_context(tc.tile_pool(name="consts", bufs=1))
        ptp = ctx.enter_context(tc.tile_pool(name="pt", bufs=6))
        normp = ctx.enter_context(tc.tile_pool(name="norm", bufs=2))
        foutp = ctx.enter_context(tc.tile_pool(name="fout", bufs=3))
        # PSUM: "s" 2x[128,1024]=4 banks, "o" 2x[65,512]=2, "b" 1, "x" 1 -> 8
        ps_s = ctx.enter_context(tc.tile_pool(name="ps_s", bufs=2, space="PSUM"))
        ps_o = ctx.enter_context(tc.tile_pool(name="ps_o", bufs=3, space="PSUM"))
        ps_x = ctx.enter_context(tc.tile_pool(name="ps_x", bufs=1, space="PSUM"))
        drp = ctx.enter_context(tc.tile_pool(name="dram", bufs=2, space="DRAM"))

        sb_xT = consts.tile([128, 4, S], BF16)  # X^T: k-chunk c -> [:, c, :]
        sb_wq = consts.tile([128, 4, DQ], BF16)
        sb_wk = consts.tile([128, 4, DQ], BF16)
        sb_wv = consts.tile([128, 4, DQ], BF16)
        sb_wo = consts.tile([128, 2, DM], BF16)  # d'-chunk p -> [:, p, :]
        sb_qT = consts.tile([128, 2, S], BF16)  # dq-chunk (head pair) p
        sb_kT = consts.tile([128, 2, S], BF16)
        sb_v = consts.tile([128, 16, HPC, 66], BF16)  # V_aug; col 64 = ones
        sb_oT = consts.tile([128, 2, S], BF16)  # normalized O^T
        sb_warm = consts.tile([128, 512], BF16)  # PE warmup fodder
        sb_one = consts.tile([128, 64], F16)  # all-ones (bcast stationary)

        nc.vector.memset(sb_one[:], 1.0)
        nc.vector.memset(sb_v[:, :, :, 64:66], 1.0)
        nc.vector.memset(sb_warm[:], 1.0)
        # wq first (QT0 needs it), then xT chunks (pace QT0's accumulation),
        # then the remaining weights (needed later than xT)
        nc.sync.dma_start(sb_wq[:], wq_d.rearrange("(c p) d -> p c d", p=128))
        xT_r = xT_d.rearrange("(c p) s -> c p s", p=128)
        for kc in range(4):
            nc.sync.dma_start(sb_xT[:, kc, :], xT_r[kc])
        for w_d, w_sb in ((wk_d, sb_wk), (wv_d, sb_wv)):
            nc.sync.dma_start(w_sb[:], w_d.rearrange("(c p) d -> p c d", p=128))
        nc.sync.dma_start(sb_wo[:], wo_d.rearrange("(c p) d -> p c d", p=128))

        # Warm the PE (HAM un-throttle needs ~3.4us of sustained matmul) and
        # preload the exp table while the xT DMA is in flight. Enough dummy
        # matmuls to keep PE busy until the DMA lands (else the MID window
        # re-throttles it right before the real work starts).
        pw = ps_x.tile([128, 512], F32, tag="x")
        for r in range(18):
            nc.tensor.matmul(
                pw[:], lhsT=sb_warm[:, 0:128], rhs=sb_warm[:], start=True, stop=True
            )
        warm_act = normp.tile([1, 4], F32, tag="wact")
        nc.scalar.activation(warm_act[:], pw[0:1, 0:4], AF.Exp, scale=-1.0)

        def emit_qk_chunk(w_sb, dst_sb, p, c, pool_tag=("ps_s", "s")):
            """One [128,512] chunk of Q^T or K^T for head-pair p."""
            isl = slice(c * 512, (c + 1) * 512)
            pool = {"ps_s": ps_s, "ps_o": ps_o, "ps_x": ps_x}[pool_tag[0]]
            pq = pool.tile([128, 512], F32, tag=pool_tag[1], name="pqk")
            for kc in range(4):
                nc.tensor.matmul(
                    pq[:],
                    lhsT=w_sb[:, kc, p * 128 : (p + 1) * 128],
                    rhs=sb_xT[:, kc, isl],
                    start=(kc == 0),
                    stop=(kc == 3),
                )
            nc.vector.tensor_copy(dst_sb[:, p, isl], pq[:])

        def emit_qk_chunk_mm(w_sb, p, c, kc, pq):
            nc.tensor.matmul(
                pq[:],
                lhsT=w_sb[:, kc, p * 128 : (p + 1) * 128],
                rhs=sb_xT[:, kc, c * 512 : (c + 1) * 512],
                start=(kc == 0),
                stop=(kc == 3),
            )

        def emit_v_chunk(sc):
            """V natural [s,dv] for s-chunk sc (all 4 heads)."""
            pv = ps_x.tile([128, DQ], F32, tag="x", name="pv")
            for kc in range(4):
                nc.tensor.matmul(
                    pv[:],
                    lhsT=sb_xT[:, kc, sc * 128 : (sc + 1) * 128],
                    rhs=sb_wv[:, kc, :],
                    start=(kc == 0),
                    stop=(kc == 3),
                )
            nc.vector.tensor_copy(
                sb_v[:, sc, :, 0:64], pv.rearrange("p (h d) -> p h d", h=HPC)
            )

        def emit_proj_chunk(c2, tag="x"):
            pf = ps_x.tile([128, 512], F32, tag=tag, name="pf") if tag == "x" else (
                ps_o.tile([128, 512], F32, tag=tag, name="pf2")
            )
            for p in range(2):
                nc.tensor.matmul(
                    pf[:],
                    lhsT=sb_oT[:, p, c2 * 128 : (c2 + 1) * 128],
                    rhs=sb_wo[:, p, :],
                    start=(p == 0),
                    stop=(p == 1),
                )
            fo = foutp.tile([128, 512], F32, tag="fo")
            nc.vector.tensor_copy(fo[:], pf[:])
            nc.sync.dma_start(out_d[c2 * 128 : (c2 + 1) * 128, :], fo[:])

        # ---- lead: Q^T (all chunks) for pair 0 + first K^T chunk; the
        # remaining K^T chunks and all V chunks stream inside block 0 ----
        for c in range(4):
            emit_qk_chunk(sb_wq, sb_qT, 0, c, ("ps_s", "s") if c % 2 else ("ps_o", "o"))
        emit_qk_chunk(sb_wk, sb_kT, 0, 0)

        # deferred work interleaved into attention blocks, one MM per j-iter
        pending_qk = []  # (w_sb, dst_sb, p, c) flattened to per-MM granularity
        for c in range(4):
            pending_qk.append((sb_wk, sb_kT, 1, c))
        for c in range(4):
            pending_qk.append((sb_wq, sb_qT, 1, c))
        qk_state = {"chunk": None, "tile": None, "kc": 0}

        def step_pending_qk():
            stt = qk_state
            if stt["chunk"] is None:
                if not pending_qk:
                    return
                stt["chunk"] = pending_qk.pop(0)
                stt["tile"] = ps_x.tile([128, 512], F32, tag="x", name="pqk1")
                stt["kc"] = 0
            w_sb, dst_sb, p, c = stt["chunk"]
            emit_qk_chunk_mm(w_sb, p, c, stt["kc"], stt["tile"])
            stt["kc"] += 1
            if stt["kc"] == 4:
                nc.vector.tensor_copy(
                    dst_sb[:, p, c * 512 : (c + 1) * 512], stt["tile"][:]
                )
                stt["chunk"] = None

        # ---- attention: pair 0 then pair 1 ----
        # Normalization of block k is emitted lazily, interleaved into the
        # first iterations of block k+1, so the in-order PE stream never
        # stalls long enough for HAM to re-throttle the clock.
        def make_norm_steps(p, ic, po):
            """Normalization of a finished block, split into 3 steps that the
            next block interleaves into its first iterations (the fp16 K=1
            broadcast matmuls sit behind fresh scores in PE order, so the PE
            never stalls waiting on the DVE sums copies)."""
            isl = slice(ic * 512, (ic + 1) * 512)
            held = {}

            def step_sums():
                for hi in range(2):
                    s = normp.tile([65, 512], F16, tag="sums", name=f"sums{hi}")
                    nc.vector.tensor_copy(s[64:65, :], po[hi][64:65, :])
                    held[hi] = s

            def step_head(hi):
                pb = ps_x.tile([64, 512], F32, tag="x", name=f"pb{hi}")
                nc.tensor.matmul(
                    pb[:],
                    lhsT=sb_one[64:65, :],
                    rhs=held[hi][64:65, :],
                    start=True,
                    stop=True,
                )
                rec = normp.tile([64, 512], F32, tag="rec", name=f"rec{hi}")
                nc.vector.reciprocal_approx_fast(rec[:], pb[:])
                if hi == 0:
                    nc.vector.tensor_mul(
                        sb_oT[0:64, p, isl], po[0][0:64, :], rec[:]
                    )
                else:
                    tmpb = normp.tile([64, 512], BF16, tag="tmpb")
                    nc.vector.tensor_mul(tmpb[:], po[1][0:64, :], rec[:])
                    nc.sync.dma_start(sb_oT[64:128, p, isl], tmpb[:])

            return [step_sums, lambda: step_head(0), lambda: step_head(1)]

        # per-MM-granularity deferred projection chunks (run during p1 blocks)
        pending_proj = []
        proj_state = {"c2": None, "tile": None, "p": 0}

        def step_pending_proj():
            stt = proj_state
            if stt["c2"] is None:
                if not pending_proj:
                    return
                stt["c2"] = pending_proj.pop(0)
                stt["tile"] = ps_x.tile([128, 512], F32, tag="x", name="pf")
                stt["p"] = 0
            c2, p = stt["c2"], stt["p"]
            nc.tensor.matmul(
                stt["tile"][:],
                lhsT=sb_oT[:, p, c2 * 128 : (c2 + 1) * 128],
                rhs=sb_wo[:, p, :],
                start=(p == 0),
                stop=(p == 1),
            )
            stt["p"] += 1
            if stt["p"] == 2:
                fo = foutp.tile([128, 512], F32, tag="fo")
                nc.vector.tensor_copy(fo[:], stt["tile"][:])
                nc.sync.dma_start(out_d[c2 * 128 : (c2 + 1) * 128, :], fo[:])
                stt["c2"] = None

        pending_norm = []
        blocks = [(p, ic) for p in range(2) for ic in range(4)]

        def emit_scores(p, ic, j):
            isl = slice(ic * 512, (ic + 1) * 512)
            jsl = slice(j * 128, (j + 1) * 128)
            st = ps_s.tile([128, 1024], F32, tag="s")
            nc.tensor.matmul(
                st[:, 0:512],
                lhsT=sb_kT[0:64, p, jsl],
                rhs=sb_qT[0:64, p, isl],
                start=True,
                stop=True,
            )
            nc.tensor.matmul(
                st[:, 512:1024],
                lhsT=sb_kT[64:128, p, jsl],
                rhs=sb_qT[64:128, p, isl],
                start=True,
                stop=True,
            )
            return st

        def emit_exp(st):
            pt = ptp.tile([128, 1024], BF16, tag="pt")
            nc.scalar.activation(pt[:], st[:], AF.Exp, scale=SCALE)
            return pt

        carry_pt = None
        for bi, (p, ic) in enumerate(blocks):
            isl = slice(ic * 512, (ic + 1) * 512)
            po = [
                ps_o.tile([65, 512], F32, tag="o", name=f"po{hi}")
                for hi in range(2)
            ]
            if p == 1 and ic > 0:
                # previous ic's projection slice; its oT inputs complete
                # during this block's first two iterations (lazy norm)
                pending_proj.extend(range(4 * (ic - 1), 4 * ic))
            pts = []  # pt tile per j (consumed by lagged AVs)
            for j in range(16):
                if j == 0 and pending_norm:
                    pending_norm[0]()  # sums copies (DVE only)
                used_carry = j == 0 and carry_pt is not None
                if used_carry:
                    pt = carry_pt  # scores+exp already ran in previous block
                    carry_pt = None
                else:
                    st = emit_scores(p, ic, j)
                if pending_norm:
                    if j == 1:
                        pending_norm[1]()  # bcast+recip+mul head 0
                    elif j == 2:
                        pending_norm[2]()  # ... head 1
                        pending_norm = []
                # extras: deferred matmuls keep PE fed; x-slot is needed
                # by the norm broadcasts at j=1,2 so extras wait till j>=3
                if p == 0 and ic == 0:
                    if 0 < j < 4:
                        emit_qk_chunk(sb_wk, sb_kT, 0, j, ("ps_o", "o"))
                    if j == 0:
                        emit_v_chunk(0)
                        emit_v_chunk(1)
                    elif j < 15:
                        emit_v_chunk(j + 1)
                elif j >= 3:
                    if p == 0:
                        step_pending_qk()
                    else:
                        step_pending_proj()
                if not used_carry:
                    pt = emit_exp(st)

                def emit_av(hi, jj, ptt):
                    nc.tensor.matmul(
                        po[hi][:],
                        lhsT=sb_v[:, jj, 2 * p + hi, 0:65],
                        rhs=ptt[:, hi * 512 : (hi + 1) * 512],
                        start=(jj == 0),
                        stop=(jj == 15),
                        skip_group_check=True,
                    )

                # uniform AV lag (h0 by 1 iter, h1 by 2) keeps scores ahead
                # of the AV stream so ACT never waits at block boundaries
                pts.append(pt)
                if j >= 1:
                    emit_av(0, j - 1, pts[j - 1])
                if j >= 2:
                    emit_av(1, j - 2, pts[j - 2])
                if j == 15:
                    emit_av(0, 15, pts[15])
                    emit_av(1, 14, pts[14])
                    emit_av(1, 15, pts[15])
                    if bi + 1 < len(blocks):
                        # cross-block pipeline: next block's first scores+exp
                        np_, nic = blocks[bi + 1]
                        carry_pt = emit_exp(emit_scores(np_, nic, 0))
            pending_norm = make_norm_steps(p, ic, po)

        # ---- tail: last normalize + remaining projection chunks ----
        for step in pending_norm:
            step()
        while pending_proj or proj_state["c2"] is not None:
            step_pending_proj()
        for c2 in range(12, 16):
            emit_proj_chunk(c2, tag="o" if c2 % 2 else "x")


def _build():
    nc = bacc.Bacc("TRN2", target_bir_lowering=False, debug=False, num_devices=N_CORES)
    xT = nc.dram_tensor("xT", [DM, S], BF16, kind="ExternalInput")
    wq = nc.dram_tensor("wq", [DM, DQ], BF16, kind="ExternalInput")
    wk = nc.dram_tensor("wk", [DM, DQ], BF16, kind="ExternalInput")
    wv = nc.dram_tensor("wv", [DM, DQ], BF16, kind="ExternalInput")
    wo = nc.dram_tensor("wo", [DQ, DM], BF16, kind="ExternalInput")
    out = nc.dram_tensor("out", [S, DM], F32, kind="ExternalOutput")
    with tile.TileContext(nc) as tc:
        _kernel_body(tc, xT.ap(), wq.ap(), wk.ap(), wv.ap(), wo.ap(), out.ap())
    nc.compile()
    return nc


def get_nc():
    global _CACHED_NC
    if _CACHED_NC is None:
        _CACHED_NC = _build()
    return _CACHED_NC


def _in_maps(hidden_states, Wq, Wk, Wv, Wo):
    bf = ml_dtypes.bfloat16
    maps = []
    for c in range(N_CORES):
        b, g = c // 2, c % 2
        cols = slice(g * DQ, (g + 1) * DQ)
        maps.append(
            {
                "xT": np.ascontiguousarray(hidden_states[b].T).astype(bf),
                "wq": np.ascontiguousarray(Wq[:, cols]).astype(bf),
                "wk": np.ascontiguousarray(Wk[:, cols]).astype(bf),
                "wv": np.ascontiguousarray(Wv[:, cols]).astype(bf),
                "wo": np.ascontiguousarray(Wo[cols, :]).astype(bf),
            }
        )
    return maps


def _ensure_profile_support():
    """Best-effort: register the axon NTFF profiling hook + defang the
    bucket upload (zero-egress container). Without this, trace=True dies
    on a missing ``antenv.axon_hooks`` module in this image."""
    import types

    try:
        import antenv

        if "antenv.axon_hooks" not in sys.modules:
            mod = types.ModuleType("antenv.axon_hooks")
            _h = {"hook": None}
            mod.set_axon_ntff_profile_hook = lambda h: _h.__setitem__("hook", h)
            mod.get_axon_ntff_profile_hook = lambda: _h["hook"]
            sys.modules["antenv.axon_hooks"] = mod
            antenv.axon_hooks = mod
        import antenv.axon_hooks as ah

        if ah.get_axon_ntff_profile_hook() is None:
            if "/root/.axon_site" not in sys.path:
                sys.path.append("/root/.axon_site")
            from trn_agent_boot.trn_boot import _ntff_profile_via_ctypes

            hook = _ntff_profile_via_ctypes("/opt/axon/libaxon_pjrt.so")
            if hook is not None:
                ah.set_axon_ntff_profile_hook(hook)
    except Exception:
        pass
    try:
        import concourse.bass_utils as bu

        bu.upload_artifacts = lambda tmpdir: tmpdir
    except Exception:
        pass


def kernel(hidden_states, Wq, Wk, Wv, Wo):
    global LAST_EXEC_TIME_NS, LAST_RESULT
    hidden_states = np.asarray(hidden_states, dtype=np.float32)
    Wq, Wk, Wv, Wo = (np.asarray(w, dtype=np.float32) for w in (Wq, Wk, Wv, Wo))

    trace = bool(os.environ.get("BASS_TRACE"))
    if trace:
        _ensure_profile_support()
    nc = get_nc()
    maps = _in_maps(hidden_states, Wq, Wk, Wv, Wo)
    res = run_bass_kernel_spmd(
        nc,
        maps,
        core_ids=list(range(N_CORES)),
        trace=trace,
        tmpdir=os.environ.get("BASS_TRACE_DIR") or None,
    )
    LAST_RESULT = res
    LAST_EXEC_TIME_NS = res.exec_time_ns

    out = np.empty((B, S, DM), dtype=np.float32)
    for b in range(B):
        out[b] = res.results[2 * b]["out"] + res.results[2 * b + 1]["out"]
    return out


if __name__ == "__main__":
    rng = np.random.default_rng(0)
    hs = rng.standard_normal((B, S, DM), dtype=np.float32)
    ws = [
        (rng.standard_normal((DM, DM), dtype=np.float32) / np.sqrt(DM))
        for _ in range(4)
    ]
    o = kernel(hs, *ws)
    print("out", o.shape, o.dtype, float(np.abs(o).mean()))
    print("exec_time_ns", LAST_EXEC_TIME_NS)
